# revision 47
# baseline (speedup 1.0000x reference)
"""Bot-detection transformer forward pass on 8 Trainium2 NeuronCores.

Strategy: data-parallel over batch (B=8 -> 1 sequence per core, no
collectives).  Residual stream kept transposed (feature-major,
xT: [768 x 1024] as 6 partition-tiles) in bf16 so projections run with
weights stationary / activations moving (full PE rate) and DVE element
ops hit the 2x 16-bit mode.

The q/k/v projections run in fp8e4 DoubleRow mode (2 contraction tiles
per matmul at 0.5 cycles/row): hT is produced in fp8e4 by the LN
normalize, the weights are scaled x32 on the host to sit in e4m3's
normal range, and the 1/32 rides the existing drain tensor_scalar ops.
The att.v matmuls also run fp8 DoubleRow (v and exp(scores) in fp8,
two key tiles contracted per pass); scores and the FFN stay bf16
(fp8 there fails the 2e-2 gate).

LayerNorm scale/bias are folded into the following projection weights
on the host, so the on-device LN is only (x-mean)*rstd; stats (sums +
sums of ACT-engine squares) ride ones-matmuls interleaved behind the
previous stage's drains, the mean/rstd row chain runs at the consumer,
and the next layer's first-half normalize is pre-computed behind the
tail of ff2 so the layer boundary starts with hT ready.  All activation
functions used live in one table (natural_log_exp_and_others); a
post-compile pass retargets the auto-inserted table loads to it and
drops ~50 redundant 1.3us reloads.

Attention: scores^T per head pair in one paired PSUM tile, one wide exp
(ACT).  v is augmented with 64 columns holding the key-padding mask, so
the att.v matmul leaves the softmax denominator replicated in PSUM rows
64:128: the drain is reciprocal + multiply (2 DVE ops), no broadcast
matmul / copies.  The av accumulation lags the score stream so the
PE never head-of-line blocks on exp.  The attention window is bound by
the ACT exp cadence (PSUM score ring depth 2), so all PE work that can
move rides inside the streams: the remaining v-projection half-tiles,
the next pair's q/k groups, and the first query-half's output
projection (inside the last pair's second-half stream).  Sprinkled
writes always precede their stream readers in program order — the tile
framework derives dependencies from program order, so a late write is
a race on hardware (CoreSim's race detector catches this).

out-proj/ff2 drain with a fused (acc+bias)+residual STT op; bias
broadcast matmuls are gone.  Weight DMAs are batched (one qkw tensor
per layer, biases packed into one [P,48] tensor, ff1 in 6 chunks, ff2
one DMA per output tile reused across both query halves) so the sync
sequencer issues ~15 descriptors per layer instead of ~90; all DRAM
layouts are pre-arranged on the host so every DMA line is >=512B
contiguous per partition.  ff1 drains on ACT (Relu with per-partition
bias ptr) since DVE is the busier engine; ff2 runs in m-pairs with the
n=0 half first so the next layer's LN1 row chain + first-half
normalize hide behind the final n=1 matmul groups.
"""

import math
from collections import deque

import numpy as np

B, S, D, H, L, V, C = 8, 1024, 768, 12, 6, 32000, 2
HD, DF, MAXPOS = 64, 3072, 2048
P = 128
KT = D // P    # 6 feature tiles
NT = S // P    # 8 token tiles
FT = DF // P   # 24 ff tiles
FCH = 6        # ff1 weight chunks
FCM = FT // FCH  # 4 m-tiles per chunk
NQ = 2         # query halves of 512
QW = S // NQ   # 512
EPS = 1e-5
N_CORES = 8

_CACHE = {}
FP8_QKV = True      # q/k/v projections in fp8e4 DoubleRow (weights x32)
FP8_SCALE = 32.0
AV_FP8 = True       # att.v in fp8e4 DoubleRow (v and exp(scores) in fp8)
AV_SCALE = 4.0 if AV_FP8 else 1.0  # v pre-scale (pow2; mask cols match)
# Optional: key-tiles whose exp runs on DVE instead of ACT via a
# Schraudolph-style direct-to-e4m3 bit trick: bits = round(s*8/ln2 + 55.5).
# Kept off: it no longer wins in the timeline sim (the score ring chain,
# not ACT throughput, limits the stream) and its HW numerics are unproven.
SCHRAUD_KTS = ()
SCHRAUD_C1 = 8.0 / math.log(2.0)
SCHRAUD_C2 = 55.5

# packed bias layout: [qkb(12) | ob(6) | f1b(24) | f2b(6)]
BQK, BOB, BF1, BF2 = 0, 12, 18, 42
NBIAS = 48


def _build_nc(n_layers=L):
    import concourse.bass as bass
    import concourse.tile as tile
    from concourse import bacc, mybir
    from concourse.bass import ds, ts
    from concourse.masks import make_identity
    from contextlib import ExitStack

    f32 = mybir.dt.float32
    bf16 = mybir.dt.bfloat16
    f32r = mybir.dt.float32r
    f8 = mybir.dt.float8e4
    u8 = mybir.dt.uint8
    i32 = mybir.dt.int32
    wdt = f8 if FP8_QKV else bf16
    vdt = f8 if AV_FP8 else bf16
    DR = mybir.MatmulPerfMode.DoubleRow
    AF = mybir.ActivationFunctionType
    OP = mybir.AluOpType

    nc = bacc.Bacc("TRN2", target_bir_lowering=False, debug=False)

    # ---------------- DRAM I/O ----------------
    d_ids = nc.dram_tensor("ids", [P, NT], i32, kind="ExternalInput")
    d_gm = nc.dram_tensor("gmask", [P, NT], f32, kind="ExternalInput")
    d_emb = nc.dram_tensor("emb", [V, D], bf16, kind="ExternalInput")
    d_posT = nc.dram_tensor("posT", [D, S], bf16, kind="ExternalInput")
    d_qkw = nc.dram_tensor("qkw", [L, P, 12, KT, P], wdt, kind="ExternalInput")
    d_vw = nc.dram_tensor("vw", [L, P, KT, D], wdt, kind="ExternalInput")
    d_vb = nc.dram_tensor("vb", [L, D], bf16, kind="ExternalInput")
    d_bias = nc.dram_tensor("biases", [L, P, NBIAS], f32, kind="ExternalInput")
    d_ow = nc.dram_tensor("ow", [L, P, KT, KT, P], bf16, kind="ExternalInput")
    d_f1w = nc.dram_tensor("f1w", [L, FCH, P, FCM, KT, P], bf16,
                           kind="ExternalInput")
    d_f2w = nc.dram_tensor("f2w", [L, KT, P, FT, P], bf16, kind="ExternalInput")
    d_cw = nc.dram_tensor("cw", [P, KT, C], f32, kind="ExternalInput")
    d_cb = nc.dram_tensor("cb", [1, C], f32, kind="ExternalInput")
    d_out = nc.dram_tensor("out", [1, C], f32, kind="ExternalOutput")

    with tile.TileContext(nc) as tc, ExitStack() as ctx:
        # ---------------- pools ----------------
        state = ctx.enter_context(tc.tile_pool(name="state", bufs=1))
        consts = ctx.enter_context(tc.tile_pool(name="consts", bufs=1))
        b24 = ctx.enter_context(tc.tile_pool(name="b24", bufs=1))
        p48 = ctx.enter_context(tc.tile_pool(name="p48", bufs=1))
        vpool = ctx.enter_context(tc.tile_pool(name="vpool", bufs=1))
        attp = ctx.enter_context(tc.tile_pool(name="attp", bufs=1))
        vwpool = ctx.enter_context(tc.tile_pool(name="vwpool", bufs=1))
        qkwp = ctx.enter_context(tc.tile_pool(name="qkwp", bufs=2))
        w6 = ctx.enter_context(tc.tile_pool(name="w6", bufs=2))
        wff2 = ctx.enter_context(tc.tile_pool(name="wff2", bufs=2))
        epool = ctx.enter_context(tc.tile_pool(name="epool", bufs=3))
        tmp = ctx.enter_context(tc.tile_pool(name="tmp", bufs=4))
        tsub = ctx.enter_context(tc.tile_pool(name="tsub", bufs=5))
        mbrb = ctx.enter_context(tc.tile_pool(name="mbrb", bufs=2))
        srows = ctx.enter_context(tc.tile_pool(name="srows", bufs=3))
        srows1 = ctx.enter_context(tc.tile_pool(name="srows1", bufs=2))
        rows = ctx.enter_context(tc.tile_pool(name="rows", bufs=2))
        params = ctx.enter_context(tc.tile_pool(name="params", bufs=2))
        # PSUM budget: 8 banks = pscore 2x2 + pmm 2x1 + patt 2x1
        pscore = ctx.enter_context(tc.tile_pool(name="pscore", bufs=2, space="PSUM"))
        pmm = ctx.enter_context(tc.tile_pool(name="pmm", bufs=2, space="PSUM"))
        patt = ctx.enter_context(tc.tile_pool(name="patt", bufs=2, space="PSUM"))

        # ---------------- constants ----------------
        xT = state.tile([P, KT, S], bf16, tag="xT")
        ones_f32 = consts.tile([P, 1], f32, tag="ones_f32")
        nc.vector.memset(ones_f32[:, :], 1.0)
        ones_rf32 = consts.tile([1, QW], f32, tag="ones_rf32")
        nc.vector.memset(ones_rf32[:, :], 1.0)
        ones_col = consts.tile([P, 1], bf16, tag="ones_col")
        nc.vector.tensor_copy(out=ones_col[:, :], in_=ones_f32[:, :])
        ones_colr = consts.tile([P, 1], f32r, tag="ones_colr")
        nc.vector.tensor_copy(out=ones_colr[:, :], in_=ones_f32[:, :])
        ones_row = consts.tile([1, QW], f32r, tag="ones_row")
        nc.vector.tensor_copy(out=ones_row[:, :], in_=ones_rf32[:, :])
        ones_rbf = consts.tile([1, QW], bf16, tag="ones_rbf")
        nc.vector.tensor_copy(out=ones_rbf[:, :], in_=ones_rf32[:, :])
        ident = consts.tile([P, P], bf16, tag="ident")
        make_identity(nc, ident[:, :])
        eps_sb = consts.tile([1, 1], f32, tag="eps")
        nc.vector.memset(eps_sb[:, :], EPS)
        ids_sb = consts.tile([P, NT], i32, tag="ids")
        nc.sync.dma_start(out=ids_sb[:, :], in_=d_ids[:, :])
        gcol = consts.tile([P, NT], f32, tag="gcol")
        nc.sync.dma_start(out=gcol[:, :], in_=d_gm[:, :])
        gcol_s = consts.tile([P, NT], f32, tag="gcol_s")
        nc.vector.tensor_scalar(
            out=gcol_s[:, :], in0=gcol[:, :],
            scalar1=(AV_SCALE / FP8_SCALE if FP8_QKV else AV_SCALE), scalar2=None,
            op0=OP.mult,
        )
        gcol_m = consts.tile([P, NT], f32, tag="gcol_m")
        nc.vector.tensor_scalar(
            out=gcol_m[:, :], in0=gcol[:, :],
            scalar1=AV_SCALE, scalar2=None, op0=OP.mult,
        )
        cw_sb = consts.tile([P, KT, C], f32r, tag="cw")
        nc.sync.dma_start(out=cw_sb[:, :, :], in_=d_cw[:, :, :].bitcast(f32r))
        cb_sb = consts.tile([1, C], f32r, tag="cb")
        nc.sync.dma_start(out=cb_sb[:, :], in_=d_cb[:, :].bitcast(f32r))

        # persistent v tile: right half holds the key-padding mask column
        # replicated 64x (written once; av matmuls then leave the softmax
        # denominator replicated in psum rows 64:128)
        v_sb = vpool.tile([P, NT, H, 2 * HD], vdt, tag="v")
        for t in range(NT):
            nc.vector.tensor_copy(
                out=v_sb[:, t, :, HD : 2 * HD],
                in_=gcol_m[:, t : t + 1].to_broadcast([P, H, HD]),
            )

        # per-layer weight prefetch (one slot ahead via bufs=2 rings)
        layer_w = {}

        def issue_layer_weights(l):
            qkw_sb = qkwp.tile([P, 12, KT, P], wdt, tag="qkw", name="qkw_sb")
            nc.sync.dma_start(out=qkw_sb[:, :, :, :], in_=d_qkw[l])
            vw_sb = qkwp.tile([P, KT, D], wdt, tag="vw", name="vw_sb")
            nc.sync.dma_start(out=vw_sb[:, :, :], in_=d_vw[l])
            vb_row = rows.tile([1, D], bf16, tag="brow")
            nc.sync.dma_start(out=vb_row[:, :], in_=d_vb[l : l + 1, :])
            bias_sb = params.tile([P, NBIAS], f32, tag="bias")
            nc.sync.dma_start(out=bias_sb[:, :], in_=d_bias[l])
            layer_w[l] = (qkw_sb, vw_sb, vb_row, bias_sb)

        issue_layer_weights(0)

        # ---------------- embedding ----------------
        posT_sb = b24.tile([P, KT, S], bf16, tag="b24")
        nc.sync.dma_start(
            out=posT_sb[:, :, :], in_=d_posT.rearrange("(j p) s -> p j s", p=P)
        )
        embts = {}

        def emb_gather(t):
            embt = tmp.tile([P, D], bf16, tag="tmp")
            nc.gpsimd.indirect_dma_start(
                out=embt[:, :],
                out_offset=None,
                in_=d_emb[:, :],
                in_offset=bass.IndirectOffsetOnAxis(ap=ids_sb[:, t : t + 1], axis=0),
            )
            embts[t] = embt

        # 3-deep prefetch: never allocate a ring slot before its previous
        # occupant's readers have been emitted
        for t in range(3):
            emb_gather(t)
        for t in range(NT):
            if t + 3 < NT:
                emb_gather(t + 3)
            embt = embts.pop(t)
            tr = pmm.tile([P, KT, P], bf16, tag="pmm")
            for j in range(KT):
                nc.tensor.transpose(
                    out=tr[:, j, :],
                    in_=embt[:, j * P : (j + 1) * P],
                    identity=ident[:, :],
                )
            nc.vector.tensor_tensor(
                out=xT[:, :, ts(t, P)], in0=tr[:, :, :],
                in1=posT_sb[:, :, ts(t, P)], op=OP.add,
            )

        # ---------------- layer norm (split into stats / finish) ----------
        def ln_stats_start():
            st = pscore.tile([1, 2 * QW], f32, tag="ps", name="st")
            return st

        def ln_stats_step(st, src, n, j):
            nsl = ds(n * QW, QW)
            sq = tmp.tile([P, QW], f32r, tag="tmp")
            nc.scalar.square(sq[:, :], src[:, j, nsl])
            nc.tensor.matmul(
                st[:, 0:QW], ones_col[:, :], src[:, j, nsl],
                start=(j == 0), stop=(j == KT - 1),
            )
            nc.tensor.matmul(
                st[:, QW : 2 * QW], ones_colr[:, :], sq[:, :],
                start=(j == 0), stop=(j == KT - 1),
            )

        def ln_rows(st):
            """Mean/rstd row chain for one query half."""
            mean = srows1.tile([1, QW], f32r, tag="mean")
            nc.vector.tensor_scalar(
                out=mean[:, :], in0=st[:, 0:QW], scalar1=1.0 / D, scalar2=None,
                op0=OP.mult,
            )
            msq = srows.tile([1, QW], f32, tag="srow")
            nc.vector.tensor_scalar(
                out=msq[:, :], in0=st[:, QW : 2 * QW], scalar1=1.0 / D,
                scalar2=None, op0=OP.mult,
            )
            var = srows.tile([1, QW], f32, tag="srow")
            nc.vector.scalar_tensor_tensor(
                out=var[:, :], in0=mean[:, :], scalar=-1.0, in1=mean[:, :],
                op0=OP.mult, op1=OP.mult,
            )
            nc.vector.tensor_tensor(
                out=var[:, :], in0=var[:, :], in1=msq[:, :], op=OP.add,
            )
            lnv = srows.tile([1, QW], f32, tag="srow")
            nc.scalar.activation(lnv[:, :], var[:, :], AF.Ln, bias=eps_sb[:, :])
            rstd = srows1.tile([1, QW], f32r, tag="rstd")
            nc.scalar.activation(rstd[:, :], lnv[:, :], AF.Exp, scale=-0.5)
            return mean, rstd

        def ln_apply(mean_rstd, src, dst, n):
            """Broadcast mean/rstd across partitions and normalize."""
            mean, rstd = mean_rstd
            nsl = ds(n * QW, QW)
            bc = pscore.tile([P, 2, QW], f32, tag="ps", name="bc")
            nc.tensor.matmul(
                bc[:, 0, :], ones_row[0:1, 0:P], mean[:, :],
                start=True, stop=True,
            )
            nc.tensor.matmul(
                bc[:, 1, :], ones_row[0:1, 0:P], rstd[:, :],
                start=True, stop=True,
            )
            mr = mbrb.tile([P, 2, QW], bf16, tag="mbrb")
            nc.scalar.copy(out=mr[:, :, :], in_=bc[:, :, :])
            for j in range(KT):
                t1 = tsub.tile([P, QW], bf16, tag="tsub")
                nc.vector.tensor_tensor(
                    out=t1[:, :], in0=src[:, j, nsl], in1=mr[:, 0, :],
                    op=OP.subtract,
                )
                nc.vector.tensor_tensor(
                    out=dst[:, j, nsl], in0=t1[:, :], in1=mr[:, 1, :],
                    op=OP.mult,
                )

        # ---------------- layers ----------------
        ln1_st = {}
        for n in range(NQ):
            st = ln_stats_start()
            for j in range(KT):
                ln_stats_step(st, xT, n, j)
            ln1_st[n] = st

        ln1_rows = {}
        hT_next = None
        for l in range(n_layers):
            qkw_sb, vw_sb, vb_row, bias_sb = layer_w.pop(l)
            hT = hT_next if hT_next is not None else b24.tile(
                [P, KT, S], wdt, tag="b24", name="hT")
            hT_next = None
            qk_sb = p48.tile([P, 12, S], bf16, tag="p48")
            attT = attp.tile([P, KT, S], bf16, tag="attT")

            def v_proj_half(t, c0, cn):
                for c0, cn in ((c0, cn),):
                    acc = pmm.tile([P, QW], f32, tag="pmm", name="vacc")
                    nc.tensor.matmul(
                        acc[:, 0:cn], ones_rbf[0:1, 0:P], vb_row[:, c0 : c0 + cn],
                        start=True, stop=False,
                    )
                    if FP8_QKV:
                        for i in range(KT // 2):
                            nc.tensor.matmul(
                                acc[:, 0:cn],
                                hT[:, 2 * i : 2 * i + 2, ts(t, P)],
                                vw_sb[:, 2 * i : 2 * i + 2, c0 : c0 + cn],
                                start=False, stop=(i == KT // 2 - 1),
                                perf_mode=DR,
                            )
                    else:
                        for j in range(KT):
                            nc.tensor.matmul(
                                acc[:, 0:cn], hT[:, j, ts(t, P)],
                                vw_sb[:, j, c0 : c0 + cn],
                                start=False, stop=(j == KT - 1),
                            )
                    nc.vector.tensor_scalar(
                        out=v_sb[:, t, c0 // HD : (c0 + cn) // HD, 0:HD],
                        in0=acc[:, 0:cn].rearrange("p (h d) -> p h d", d=HD),
                        scalar1=gcol_s[:, t : t + 1], scalar2=None, op0=OP.mult,
                    )

            def qk_group(m, n):
                nsl = ds(n * QW, QW)
                acc = pmm.tile([P, QW], f32, tag="pmm", name="qkacc")
                if FP8_QKV:
                    for i in range(KT // 2):
                        nc.tensor.matmul(
                            acc[:, :],
                            qkw_sb[:, m, 2 * i : 2 * i + 2, :],
                            hT[:, 2 * i : 2 * i + 2, nsl],
                            start=(i == 0), stop=(i == KT // 2 - 1),
                            perf_mode=DR,
                        )
                    nc.vector.tensor_scalar(
                        out=qk_sb[:, m, nsl], in0=acc[:, :],
                        scalar1=1.0 / FP8_SCALE,
                        scalar2=bias_sb[:, BQK + m : BQK + m + 1],
                        op0=OP.mult, op1=OP.add,
                    )
                else:
                    for j in range(KT):
                        nc.tensor.matmul(
                            acc[:, :], qkw_sb[:, m, j, :], hT[:, j, nsl],
                            start=(j == 0), stop=(j == KT - 1),
                        )
                    nc.vector.tensor_scalar(
                        out=qk_sb[:, m, nsl], in0=acc[:, :],
                        scalar1=bias_sb[:, BQK + m : BQK + m + 1], scalar2=None,
                        op0=OP.add,
                    )

            # LN1 finish + minimal serial prefix: only pair-0 q/k and the
            # first two v tiles run before the score streams; the remaining
            # v tiles and later pairs' q/k ride inside the streams (the PE
            # has slack there — the streams are ACT-exp-bound).
            def v_proj_t(t):
                v_proj_half(t, 0, QW)
                v_proj_half(t, QW, D - QW)

            if 0 in ln1_st:
                ln1_rows[0] = ln_rows(ln1_st.pop(0))
            if 0 in ln1_rows:
                ln_apply(ln1_rows.pop(0), xT, hT, 0)
            # n=0-half work first: hT n=0 is ready from the previous layer's
            # tail, so PE starts immediately; the n=1 row chain + normalize
            # run on ACT/DVE behind it
            v_proj_t(0)
            v_proj_t(1)
            qk_group(6, 0)
            qk_group(0, 0)
            if 1 in ln1_st:
                ln1_rows[1] = ln_rows(ln1_st.pop(1))
            if 1 in ln1_rows:
                ln_apply(ln1_rows.pop(1), xT, hT, 1)
            v_proj_t(2)
            v_proj_t(3)
            qk_group(6, 1)
            qk_group(0, 1)
            vq_queue = []
            for t in range(4, NT):
                vq_queue.append(("v", t, 0, QW))
                vq_queue.append(("v", t, QW, D - QW))

            # out-proj weights prefetched behind the attention stream
            ow_sb = vwpool.tile([P, KT, KT, P], bf16, tag="ow")
            nc.sync.dma_start(out=ow_sb[:, :, :, :], in_=d_ow[l])

            ln2_st = {}

            def out_proj_half(n, with_stats):
                nsl = ds(n * QW, QW)
                st2 = ln_stats_start() if with_stats else None
                for m in range(KT):
                    acc = pmm.tile([P, QW], f32, tag="pmm", name="oacc")
                    for j in range(KT):
                        nc.tensor.matmul(
                            acc[:, :], ow_sb[:, m, j, :], attT[:, j, nsl],
                            start=(j == 0), stop=(j == KT - 1),
                        )
                    nc.vector.scalar_tensor_tensor(
                        out=xT[:, m, nsl], in0=acc[:, :],
                        scalar=bias_sb[:, BOB + m : BOB + m + 1],
                        in1=xT[:, m, nsl],
                        op0=OP.add, op1=OP.add,
                    )
                    if with_stats:
                        ln_stats_step(st2, xT, n, m)
                if with_stats:
                    ln2_st[n] = st2

            for hp in range(H // 2):
                hA, hB = 2 * hp, 2 * hp + 1
                if hp + 1 < H // 2:
                    # k groups first: scores kt spans both halves of k
                    vq_queue += [("qk", 7 + hp, 0), ("qk", 7 + hp, 1),
                                 ("qk", 1 + hp, 0), ("qk", 1 + hp, 1)]
                pend = deque()
                pats = {}

                if AV_FP8:
                    def av_emit(n, ktp, epair):
                        if ktp == 0:
                            patA = patt.tile([P, QW], f32, tag="patt", name="patA")
                            patB = patt.tile([P, QW], f32, tag="patt", name="patB")
                            pats[n] = (patA, patB)
                        pA, pB = pats[n]
                        nc.tensor.matmul(
                            pA[:, :], v_sb[:, 2 * ktp : 2 * ktp + 2, hA, :],
                            epair[:, :, 0:QW],
                            start=(ktp == 0), stop=(ktp == NT // 2 - 1),
                            perf_mode=DR,
                        )
                        nc.tensor.matmul(
                            pB[:, :], v_sb[:, 2 * ktp : 2 * ktp + 2, hB, :],
                            epair[:, :, QW : 2 * QW],
                            start=(ktp == 0), stop=(ktp == NT // 2 - 1),
                            perf_mode=DR,
                        )
                else:
                    def av_emit(n, kt, e):
                        if kt == 0:
                            patA = patt.tile([P, QW], f32, tag="patt", name="patA")
                            patB = patt.tile([P, QW], f32, tag="patt", name="patB")
                            pats[n] = (patA, patB)
                        pA, pB = pats[n]
                        nc.tensor.matmul(
                            pA[:, :], v_sb[:, kt, hA, :], e[:, 0:QW],
                            start=(kt == 0), stop=(kt == NT - 1),
                        )
                        nc.tensor.matmul(
                            pB[:, :], v_sb[:, kt, hB, :], e[:, QW : 2 * QW],
                            start=(kt == 0), stop=(kt == NT - 1),
                        )

                for n in range(NQ):
                    nsl = ds(n * QW, QW)
                    epair = None
                    for kt in range(NT):
                        ps = pscore.tile([P, 2 * QW], f32, tag="ps")
                        nc.tensor.matmul(
                            ps[:, 0:QW],
                            qk_sb[0:HD, 6 + hp, ts(kt, P)],
                            qk_sb[0:HD, hp, nsl],
                            start=True, stop=True,
                        )
                        nc.tensor.matmul(
                            ps[:, QW : 2 * QW],
                            qk_sb[HD:P, 6 + hp, ts(kt, P)],
                            qk_sb[HD:P, hp, nsl],
                            start=True, stop=True,
                        )
                        if AV_FP8:
                            if kt % 2 == 0:
                                epair = epool.tile([P, 2, 2 * QW], f8, tag="e",
                                                   name="epair")
                            if kt in SCHRAUD_KTS:
                                nc.vector.tensor_scalar(
                                    out=epair[:, kt % 2, :].bitcast(u8),
                                    in0=ps[:, :], scalar1=SCHRAUD_C1,
                                    scalar2=SCHRAUD_C2,
                                    op0=OP.mult, op1=OP.add,
                                )
                            else:
                                nc.scalar.activation(
                                    epair[:, kt % 2, :], ps[:, :], AF.Exp)
                            if kt % 2 == 1:
                                pend.append((n, kt // 2, epair))
                                if len(pend) >= 2:
                                    av_emit(*pend.popleft())
                        else:
                            e = epool.tile([P, 2 * QW], bf16, tag="e")
                            nc.scalar.activation(e[:, :], ps[:, :], AF.Exp)
                            pend.append((n, kt, e))
                            if len(pend) >= 3:
                                av_emit(*pend.popleft())
                        if 1 <= kt <= 6 and vq_queue:
                            # two pops when both are v-halves: the first
                            # stream must emit every v write before the av
                            # that reads it (program order = dependency
                            # order for the tile framework)
                            for _ in range(2 if vq_queue[0][0] == "v" else 1):
                                if not vq_queue:
                                    break
                                task = vq_queue.pop(0)
                                if task[0] == "v":
                                    v_proj_half(task[1], task[2], task[3])
                                else:
                                    qk_group(task[1], task[2])
                    while pend:
                        av_emit(*pend.popleft())
                    # drain: denominator is replicated in psum rows 64:128
                    pA, pB = pats.pop(n)
                    for pat, po in ((pA, 0), (pB, HD)):
                        zinv = srows.tile([HD, QW], bf16, tag="zinv")
                        with nc.allow_low_precision(reason="softmax denom bf16"):
                            nc.vector.reciprocal(zinv[:, :], pat[HD:P, :])
                        nc.vector.tensor_tensor(
                            out=attT[po : po + HD, hp, nsl],
                            in0=pat[0:HD, :], in1=zinv[:, :], op=OP.mult,
                        )
                    if hp == H // 2 - 1 and n == 0:
                        # n=0 attention fully drained: overlap the n=0
                        # out-projection with the last pair's n=1 stream
                        out_proj_half(0, with_stats=False)
                while vq_queue:
                    task = vq_queue.pop(0)
                    if task[0] == "v":
                        v_proj_half(task[1], task[2], task[3])
                    else:
                        qk_group(task[1], task[2])

            # ---- output projection + residual, n-split, LN2 stats behind;
            # the n=0 half was emitted inside the last head pair's n=1
            # attention stream ----
            st2 = ln_stats_start()
            for m in range(KT):
                ln_stats_step(st2, xT, 0, m)
            ln2_st[0] = st2
            # n=0 row chain + normalize issued before out_proj(1) so all
            # their ACT/DVE latency hides behind those matmuls and ff1 can
            # start the moment out_proj(1) drains
            r0 = ln_rows(ln2_st.pop(0))
            h2 = b24.tile([P, KT, S], bf16, tag="b24")
            ln_apply(r0, xT, h2, 0)
            out_proj_half(1, with_stats=True)
            r1 = ln_rows(ln2_st.pop(1))
            ln_apply(r1, xT, h2, 1)

            f1w_tiles = {}

            def dma_f1w(ch):
                wt = w6.tile([P, FCM, KT, P], bf16, tag="w6", name="f1wc")
                nc.sync.dma_start(out=wt[:, :, :, :], in_=d_f1w[l, ch])
                f1w_tiles[ch] = wt

            f2w_tiles = {}

            def dma_f2w(m):
                w2 = wff2.tile([P, FT, P], bf16, tag="wff2", name="f2wt")
                nc.sync.dma_start(out=w2[:, :, :], in_=d_f2w[l, m])
                f2w_tiles[m] = w2

            dma_f1w(0)
            dma_f1w(1)
            if l + 1 < n_layers:
                issue_layer_weights(l + 1)
            dma_f2w(0)
            dma_f2w(1)

            f_sb = p48.tile([P, FT, S], bf16, tag="p48")
            for ch in range(FCH):
                if ch >= 1 and ch + 1 < FCH:
                    dma_f1w(ch + 1)
                wt = f1w_tiles.pop(ch)
                # n-major within the chunk: the first chunk's n=0 groups can
                # start as soon as the n=0 normalize lands, hiding the n=1
                # apply latency behind them
                for n in range(NQ):
                    for mi in range(FCM):
                        m = ch * FCM + mi
                        nsl = ds(n * QW, QW)
                        acc = pmm.tile([P, QW], f32, tag="pmm", name="facc")
                        for j in range(KT):
                            nc.tensor.matmul(
                                acc[:, :], wt[:, mi, j, :], h2[:, j, nsl],
                                start=(j == 0), stop=(j == KT - 1),
                            )
                        # drain on ACT (idle during FFN; DVE is the busy one)
                        nc.scalar.activation(
                            f_sb[:, m, nsl], acc[:, :], AF.Relu,
                            bias=bias_sb[:, BF1 + m : BF1 + m + 1],
                        )

            last = l == n_layers - 1
            st1 = {} if last else {n: ln_stats_start() for n in range(NQ)}

            def ff2_half(m, n):
                nsl = ds(n * QW, QW)
                w2 = f2w_tiles[m]
                acc = pmm.tile([P, QW], f32, tag="pmm", name="f2acc")
                for j in range(FT):
                    nc.tensor.matmul(
                        acc[:, :], w2[:, j, :], f_sb[:, j, nsl],
                        start=(j == 0), stop=(j == FT - 1),
                    )
                nc.vector.scalar_tensor_tensor(
                    out=xT[:, m, nsl], in0=acc[:, :],
                    scalar=bias_sb[:, BF2 + m : BF2 + m + 1],
                    in1=xT[:, m, nsl],
                    op0=OP.add, op1=OP.add,
                )
                if not last:
                    ln_stats_step(st1[n], xT, n, m)

            # m-pairs with n=0 first within each pair: st1[0] closes two
            # groups before the end, so the next layer's first-half
            # normalize hides behind the final n=1 groups
            order = [(0, 0), (1, 0), (0, 1), (1, 1), (2, 0), (3, 0),
                     (2, 1), (3, 1), (4, 0), (5, 0), (4, 1), (5, 1)]
            done_n = {m: 0 for m in range(KT)}
            for m, n in order:
                ff2_half(m, n)
                done_n[m] += 1
                if done_n[m] == NQ:
                    f2w_tiles.pop(m)
                    if m + 2 < KT:
                        dma_f2w(m + 2)
                if (m, n) == (KT - 1, 0) and not last:
                    ln1_rows[0] = ln_rows(st1.pop(0))
                    hT_next = b24.tile([P, KT, S], wdt, tag="b24", name="hTn")
                    ln_apply(ln1_rows.pop(0), xT, hT_next, 0)
            if not last:
                ln1_st[1] = st1[1]

        # ---------------- CLS head ----------------
        col2 = xT[:, :, 0:2]  # (P, KT, 2) bf16
        xsqc = consts.tile([P, KT, 2], bf16, tag="xsqc")
        nc.scalar.square(xsqc[:, :, :], col2)
        pss = pmm.tile([1, QW], f32, tag="pmm")
        for j in range(KT):
            nc.tensor.matmul(
                pss[:, 0:2], ones_col[:, :], xT[:, j, 0:2],
                start=(j == 0), stop=(j == KT - 1),
            )
        for j in range(KT):
            nc.tensor.matmul(
                pss[:, 2:4], ones_col[:, :], xsqc[:, j, :],
                start=(j == 0), stop=(j == KT - 1),
            )
        hmean = srows1.tile([1, 64], f32r, tag="mean")
        nc.vector.tensor_scalar(
            out=hmean[:, 0:2], in0=pss[:, 0:2], scalar1=1.0 / D, scalar2=None,
            op0=OP.mult,
        )
        hmsq = srows.tile([1, 64], f32, tag="srow")
        nc.vector.tensor_scalar(
            out=hmsq[:, 0:2], in0=pss[:, 2:4], scalar1=1.0 / D, scalar2=None,
            op0=OP.mult,
        )
        hvar = srows.tile([1, 64], f32, tag="srow")
        nc.vector.scalar_tensor_tensor(
            out=hvar[:, 0:2], in0=hmean[:, 0:2], scalar=-1.0, in1=hmean[:, 0:2],
            op0=OP.mult, op1=OP.mult,
        )
        nc.vector.tensor_tensor(
            out=hvar[:, 0:2], in0=hvar[:, 0:2], in1=hmsq[:, 0:2], op=OP.add
        )
        hlnv = srows.tile([1, 64], f32, tag="srow")
        nc.scalar.activation(hlnv[:, 0:2], hvar[:, 0:2], AF.Ln, bias=eps_sb[:, :])
        hrstd = srows1.tile([1, 64], f32r, tag="rstd")
        nc.scalar.activation(hrstd[:, 0:2], hlnv[:, 0:2], AF.Exp, scale=-0.5)
        pbc = pmm.tile([P, QW], f32, tag="pmm")
        nc.tensor.matmul(pbc[:, 0:2], ones_row[0:1, 0:P], hmean[:, 0:2],
                         start=True, stop=True)
        nc.tensor.matmul(pbc[:, 2:4], ones_row[0:1, 0:P], hrstd[:, 0:2],
                         start=True, stop=True)
        t1 = consts.tile([P, KT, 2], f32, tag="ht1")
        nc.vector.tensor_tensor(
            out=t1[:, :, :], in0=col2, in1=pbc[:, 0:1].to_broadcast([P, KT, 2]),
            op=OP.subtract,
        )
        pc = consts.tile([P, KT, 2], f32r, tag="pc")
        nc.vector.tensor_tensor(
            out=pc[:, :, :], in0=t1[:, :, :], in1=pbc[:, 2:3].to_broadcast([P, KT, 2]),
            op=OP.mult,
        )
        plog = patt.tile([P, QW], f32, tag="patt")
        nc.tensor.matmul(
            plog[0:C, 0:2], cb_sb[:, :], ones_row[:, 0:2], start=True, stop=False
        )
        for j in range(KT):
            nc.tensor.matmul(
                plog[0:C, 0:2], cw_sb[:, j, :], pc[:, j, :],
                start=False, stop=(j == KT - 1),
            )
        out_sb = consts.tile([C, 1], f32, tag="outsb")
        nc.vector.tensor_copy(out=out_sb[:, :], in_=plog[0:C, 0:1])
        nc.sync.dma_start(out=d_out[0:1, 0:C], in_=out_sb[0:C, 0:1])

    nc.compile()
    _patch_act_tables(nc)
    return nc


def _patch_act_tables(nc):
    """All activation functions used here (exp, ln, square, copy, relu,
    identity) live together in act func set 6 (natural_log_exp_and_others),
    but the table-load pass picks the first set containing each function,
    inserting ~50 1.3us reloads.  Retarget the first load to set 6 and drop
    the redundant ones (they carry no semaphore info)."""
    from concourse import mybir

    first = True
    for b in nc.m.functions[0].blocks:
        keep = []
        for inst in b.instructions:
            if isinstance(inst, mybir.InstLoadActFuncSet):
                si = inst.sync_info
                has_sems = si is not None and (
                    len(si.on_wait) > 0 or len(si.on_update) > 0
                )
                if first or has_sems:
                    inst.act_func_set_id = 6
                    keep.append(inst)
                    first = False
            else:
                keep.append(inst)
        b.instructions[:] = keep


def _bf16np():
    import ml_dtypes

    return ml_dtypes.bfloat16


def _f8np():
    import ml_dtypes

    return ml_dtypes.float8_e4m3fn


def _prep_host(inputs):
    g = lambda k: np.asarray(inputs[k])
    bf = _bf16np()
    sq = np.float32(math.sqrt(D))
    ids = g("input_ids").astype(np.int32)              # (B, S)
    gm = (1.0 - g("attention_mask").astype(np.float32))  # (B, S)
    emb = (g("token_emb").astype(np.float32) * sq)
    posT = np.ascontiguousarray((g("pos_emb")[:S].astype(np.float32) * sq).T)
    # reference reshapes qkv output to (H, 3, HD): permute columns into
    # contiguous q | k | v blocks (each h-major) before tiling
    idx = np.arange(3 * D).reshape(H, 3, HD)
    cols = np.concatenate(
        [idx[:, 0, :].reshape(-1), idx[:, 1, :].reshape(-1), idx[:, 2, :].reshape(-1)]
    )
    qkv_w_orig = g("qkv_w").astype(np.float32)[:, :, cols].copy()  # (L, D, 3D)
    qkv_b = g("qkv_b").astype(np.float32)[:, cols].copy()          # (L, 3D)
    qkv_w_orig[:, :, :D] *= np.float32(1.0 / math.sqrt(HD))
    qkv_b[:, :D] *= np.float32(1.0 / math.sqrt(HD))
    # fold LN1 scale/bias into qkv:  (x*s+b) @ W = x @ (diag(s)W) + (b@W)
    n1_s = g("n1_s").astype(np.float32)   # (L, D)
    n1_b = g("n1_b").astype(np.float32)
    qkv_w = qkv_w_orig * n1_s[:, :, None]
    qkv_b = qkv_b + np.einsum("ld,lde->le", n1_b, qkv_w_orig)
    # fold LN2 into ff1
    n2_s = g("n2_s").astype(np.float32)
    n2_b = g("n2_b").astype(np.float32)
    ff1_w_orig = g("ff1_w").astype(np.float32)          # (L, D, DF)
    ff1_w = ff1_w_orig * n2_s[:, :, None]
    ff1_b = g("ff1_b").astype(np.float32) + np.einsum("ld,ldf->lf", n2_b, ff1_w_orig)
    # fold head LN into cls
    hln_s = g("hln_s").astype(np.float32)
    hln_b = g("hln_b").astype(np.float32)
    cls_w_orig = g("cls_w").astype(np.float32)          # (D, C)
    cls_w = cls_w_orig * hln_s[:, None]
    cls_b = g("cls_b").astype(np.float32) + hln_b @ cls_w_orig

    wnp = _f8np() if FP8_QKV else bf
    qw_scale = np.float32(FP8_SCALE) if FP8_QKV else np.float32(1.0)
    # packed per-layer biases: [qkb(12) | ob(6) | f1b(24) | f2b(6)] as [L,P,48]
    qkb_t = qkv_b[:, : 2 * D].reshape(L, 12, P).transpose(0, 2, 1)
    ob_t = g("out_b").astype(np.float32).reshape(L, KT, P).transpose(0, 2, 1)
    f1b_t = ff1_b.reshape(L, FT, P).transpose(0, 2, 1)
    f2b_t = g("ff2_b").astype(np.float32).reshape(L, KT, P).transpose(0, 2, 1)
    biases = np.ascontiguousarray(
        np.concatenate([qkb_t, ob_t, f1b_t, f2b_t], axis=2)
    )
    shared = {
        "emb": emb.astype(bf),
        "posT": posT.astype(bf),
        "qkw": np.ascontiguousarray(
            (qkv_w[:, :, : 2 * D] * qw_scale)
            .reshape(L, KT, P, 12, P).transpose(0, 2, 3, 1, 4)
        ).astype(wnp),
        "vw": np.ascontiguousarray(
            (qkv_w[:, :, 2 * D :] * qw_scale)
            .reshape(L, KT, P, D).transpose(0, 2, 1, 3)
        ).astype(wnp),
        "vb": np.ascontiguousarray(qkv_b[:, 2 * D :] * qw_scale).astype(bf),
        "biases": biases,
        "ow": np.ascontiguousarray(
            g("out_w").astype(np.float32).reshape(L, KT, P, KT, P).transpose(0, 2, 3, 1, 4)
        ).astype(bf),
        "f1w": np.ascontiguousarray(
            ff1_w.reshape(L, KT, P, FT, P).transpose(0, 3, 2, 1, 4)
            .reshape(L, FCH, FCM, P, KT, P).transpose(0, 1, 3, 2, 4, 5)
        ).astype(bf),
        "f2w": np.ascontiguousarray(
            g("ff2_w").astype(np.float32).reshape(L, FT, P, KT, P).transpose(0, 3, 2, 1, 4)
        ).astype(bf),
        "cw": np.ascontiguousarray(cls_w.reshape(KT, P, C).transpose(1, 0, 2)),
        "cb": cls_b.reshape(1, C),
    }
    per_core = []
    for c in range(N_CORES):
        per_core.append(
            {
                "ids": np.ascontiguousarray(ids[c].reshape(NT, P).T),
                "gmask": np.ascontiguousarray(gm[c].reshape(NT, P).T),
            }
        )
    return shared, per_core


def _get_nc():
    if "nc" not in _CACHE:
        _CACHE["nc"] = _build_nc()
    return _CACHE["nc"]


def kernel(**inputs):
    from concourse.bass_utils import run_bass_kernel_spmd

    shared, per_core = _prep_host(inputs)
    nc = _get_nc()
    in_maps = [dict(shared, **per_core[c]) for c in range(N_CORES)]
    _CACHE["in_maps"] = in_maps
    res = run_bass_kernel_spmd(nc, in_maps, list(range(N_CORES)))
    out = np.stack([res.results[c]["out"][0] for c in range(N_CORES)], axis=0)
    return out.astype(np.float32)


def bench(n_iters=10):
    """Re-run the compiled NEFF with device-resident inputs; returns the
    best-observed per-iteration wall time in ns (upper bound on HW exec)."""
    import time

    import jax
    import numpy as _np
    from jax.sharding import Mesh, PartitionSpec, NamedSharding
    from jax.experimental.shard_map import shard_map
    from concourse import bass2jax, mybir
    from concourse.bass2jax import _bass_exec_p, install_neuronx_cc_hook

    nc = _get_nc()
    in_maps = _CACHE["in_maps"]
    install_neuronx_cc_hook()

    pname = nc.partition_id_tensor.name if nc.partition_id_tensor else None
    in_names, out_names, out_avals, zero_outs = [], [], [], []
    for alloc in nc.m.functions[0].allocations:
        if not isinstance(alloc, mybir.MemoryLocationSet):
            continue
        name = alloc.memorylocations[0].name
        if alloc.kind == "ExternalInput":
            if name == pname:
                continue
            in_names.append(name)
        elif alloc.kind == "ExternalOutput":
            out_names.append(name)
            shape = tuple(alloc.tensor_shape)
            dtype = mybir.dt.np(alloc.dtype)
            out_avals.append(jax.core.ShapedArray(shape, dtype))
            zero_outs.append(_np.zeros(shape, dtype))
    n_params = len(in_names)
    all_names = in_names + out_names
    if pname is not None:
        all_names = all_names + [pname]

    def _body(*args):
        operands = list(args)
        if pname is not None:
            operands.append(bass2jax.partition_id_tensor())
        outs = _bass_exec_p.bind(
            *operands,
            out_avals=tuple(out_avals),
            in_names=tuple(all_names),
            out_names=tuple(out_names),
            lowering_input_output_aliases=(),
            sim_require_finite=True,
            sim_require_nnan=True,
            nc=nc,
        )
        return tuple(outs)

    devices = jax.devices()[:N_CORES]
    mesh = Mesh(_np.asarray(devices), ("core",))
    nin = n_params + len(zero_outs)
    fn = jax.jit(
        shard_map(
            _body,
            mesh=mesh,
            in_specs=(PartitionSpec("core"),) * nin,
            out_specs=(PartitionSpec("core"),) * len(out_names),
            check_rep=False,
        )
    )
    sharding = NamedSharding(mesh, PartitionSpec("core"))
    concat_in = [
        jax.device_put(
            _np.concatenate([_np.asarray(in_maps[c][n]) for c in range(N_CORES)], 0),
            sharding,
        )
        for n in in_names
    ]
    concat_zeros = [
        jax.device_put(
            _np.zeros((N_CORES * z.shape[0], *z.shape[1:]), z.dtype), sharding
        )
        for z in zero_outs
    ]
    jax.block_until_ready(concat_in)
    # warmup (compile)
    out = fn(*concat_in, *concat_zeros)
    jax.block_until_ready(out)
    # pipelined async dispatch amortizes the axon tunnel round-trip
    outs = []
    t0 = time.perf_counter()
    for _ in range(n_iters):
        outs.append(fn(*concat_in, *concat_zeros))
    jax.block_until_ready(outs)
    dt = (time.perf_counter() - t0) / n_iters
    return int(dt * 1e9)


# revision 48
# speedup vs baseline: 1.6209x; 1.6209x over previous
"""Bot-detection transformer forward pass on 8 Trainium2 NeuronCores.

Strategy: data-parallel over batch (B=8 -> 1 sequence per core, no
collectives).  Residual stream kept transposed (feature-major,
xT: [768 x 1024] as 6 partition-tiles) in bf16 so projections run with
weights stationary / activations moving (full PE rate) and DVE element
ops hit the 2x 16-bit mode.

The q/k/v projections run in fp8e4 DoubleRow mode (2 contraction tiles
per matmul at 0.5 cycles/row): hT is produced in fp8e4 by the LN
normalize, the weights are scaled x32 on the host to sit in e4m3's
normal range, and the 1/32 rides the existing drain tensor_scalar ops.
The att.v matmuls also run fp8 DoubleRow (v and exp(scores) in fp8,
two key tiles contracted per pass); scores and the FFN stay bf16
(fp8 there fails the 2e-2 gate).

LayerNorm scale/bias are folded into the following projection weights
on the host, so the on-device LN is only (x-mean)*rstd; stats (sums +
sums of ACT-engine squares) ride ones-matmuls interleaved behind the
previous stage's drains, the mean/rstd row chain runs at the consumer,
and the next layer's first-half normalize is pre-computed behind the
tail of ff2 so the layer boundary starts with hT ready.  All activation
functions used live in one table (natural_log_exp_and_others); a
post-compile pass retargets the auto-inserted table loads to it and
drops ~50 redundant 1.3us reloads.

Attention: scores^T per head pair in one paired PSUM tile, one wide exp
(ACT).  v is augmented with 64 columns holding the key-padding mask, so
the att.v matmul leaves the softmax denominator replicated in PSUM rows
64:128: the drain is reciprocal + multiply (2 DVE ops), no broadcast
matmul / copies.  The av accumulation lags the score stream so the
PE never head-of-line blocks on exp.  The attention window is bound by
the ACT exp cadence (PSUM score ring depth 2), so all PE work that can
move rides inside the streams: the remaining v-projection half-tiles,
the next pair's q/k groups, and the first query-half's output
projection (inside the last pair's second-half stream).  Sprinkled
writes always precede their stream readers in program order — the tile
framework derives dependencies from program order, so a late write is
a race on hardware (CoreSim's race detector catches this).

out-proj/ff2 drain with a fused (acc+bias)+residual STT op; bias
broadcast matmuls are gone.  Weight DMAs are batched (one qkw tensor
per layer, biases packed into one [P,48] tensor, ff1 in 6 chunks, ff2
one DMA per output tile reused across both query halves) so the sync
sequencer issues ~15 descriptors per layer instead of ~90; all DRAM
layouts are pre-arranged on the host so every DMA line is >=512B
contiguous per partition.  ff1 drains on ACT (Relu with per-partition
bias ptr) since DVE is the busier engine; ff2 runs in m-pairs with the
n=0 half first so the next layer's LN1 row chain + first-half
normalize hide behind the final n=1 matmul groups.
"""

import math
from collections import deque

import numpy as np

B, S, D, H, L, V, C = 8, 1024, 768, 12, 6, 32000, 2
HD, DF, MAXPOS = 64, 3072, 2048
P = 128
KT = D // P    # 6 feature tiles
NT = S // P    # 8 token tiles
FT = DF // P   # 24 ff tiles
FCH = 6        # ff1 weight chunks
FCM = FT // FCH  # 4 m-tiles per chunk
NQ = 2         # query halves of 512
QW = S // NQ   # 512
EPS = 1e-5
N_CORES = 8

_CACHE = {}
FP8_QKV = True      # q/k/v projections in fp8e4 DoubleRow (weights x32)
FP8_SCALE = 32.0
AV_FP8 = True       # att.v in fp8e4 DoubleRow (v and exp(scores) in fp8)
AV_SCALE = 4.0 if AV_FP8 else 1.0  # v pre-scale (pow2; mask cols match)
# Key-tiles whose exp runs on DVE instead of ACT via a Schraudolph-style
# direct-to-e4m3 bit trick: bits = round(s*8/ln2 + 55.5), written as uint8
# and bitcast to fp8e4.  Offloads 3/8 of the exp stream from the ACT
# bottleneck, and measures *better* on hardware than ACT exp + f8 cast
# (8.6e-3 vs 1.5e-2 rel err — the DVE f32->u8 convert rounds to nearest
# while the ACT f8 store appears to truncate).
SCHRAUD_KTS = (2, 5, 7)
SCHRAUD_C1 = 8.0 / math.log(2.0)
SCHRAUD_C2 = 55.5

# packed bias layout: [qkb(12) | ob(6) | f1b(24) | f2b(6)]
BQK, BOB, BF1, BF2 = 0, 12, 18, 42
NBIAS = 48


def _build_nc(n_layers=L):
    import concourse.bass as bass
    import concourse.tile as tile
    from concourse import bacc, mybir
    from concourse.bass import ds, ts
    from concourse.masks import make_identity
    from contextlib import ExitStack

    f32 = mybir.dt.float32
    bf16 = mybir.dt.bfloat16
    f32r = mybir.dt.float32r
    f8 = mybir.dt.float8e4
    u8 = mybir.dt.uint8
    i32 = mybir.dt.int32
    wdt = f8 if FP8_QKV else bf16
    vdt = f8 if AV_FP8 else bf16
    DR = mybir.MatmulPerfMode.DoubleRow
    AF = mybir.ActivationFunctionType
    OP = mybir.AluOpType

    nc = bacc.Bacc("TRN2", target_bir_lowering=False, debug=False)

    # ---------------- DRAM I/O ----------------
    d_ids = nc.dram_tensor("ids", [P, NT], i32, kind="ExternalInput")
    d_gm = nc.dram_tensor("gmask", [P, NT], f32, kind="ExternalInput")
    d_emb = nc.dram_tensor("emb", [V, D], bf16, kind="ExternalInput")
    d_posT = nc.dram_tensor("posT", [D, S], bf16, kind="ExternalInput")
    d_qkw = nc.dram_tensor("qkw", [L, P, 12, KT, P], wdt, kind="ExternalInput")
    d_vw = nc.dram_tensor("vw", [L, P, KT, D], wdt, kind="ExternalInput")
    d_vb = nc.dram_tensor("vb", [L, D], bf16, kind="ExternalInput")
    d_bias = nc.dram_tensor("biases", [L, P, NBIAS], f32, kind="ExternalInput")
    d_ow = nc.dram_tensor("ow", [L, P, KT, KT, P], bf16, kind="ExternalInput")
    d_f1w = nc.dram_tensor("f1w", [L, FCH, P, FCM, KT, P], bf16,
                           kind="ExternalInput")
    d_f2w = nc.dram_tensor("f2w", [L, KT, P, FT, P], bf16, kind="ExternalInput")
    d_cw = nc.dram_tensor("cw", [P, KT, C], f32, kind="ExternalInput")
    d_cb = nc.dram_tensor("cb", [1, C], f32, kind="ExternalInput")
    d_out = nc.dram_tensor("out", [1, C], f32, kind="ExternalOutput")

    with tile.TileContext(nc) as tc, ExitStack() as ctx:
        # ---------------- pools ----------------
        state = ctx.enter_context(tc.tile_pool(name="state", bufs=1))
        consts = ctx.enter_context(tc.tile_pool(name="consts", bufs=1))
        b24 = ctx.enter_context(tc.tile_pool(name="b24", bufs=1))
        p48 = ctx.enter_context(tc.tile_pool(name="p48", bufs=1))
        vpool = ctx.enter_context(tc.tile_pool(name="vpool", bufs=1))
        attp = ctx.enter_context(tc.tile_pool(name="attp", bufs=1))
        vwpool = ctx.enter_context(tc.tile_pool(name="vwpool", bufs=1))
        qkwp = ctx.enter_context(tc.tile_pool(name="qkwp", bufs=2))
        w6 = ctx.enter_context(tc.tile_pool(name="w6", bufs=2))
        wff2 = ctx.enter_context(tc.tile_pool(name="wff2", bufs=2))
        epool = ctx.enter_context(tc.tile_pool(name="epool", bufs=3))
        tmp = ctx.enter_context(tc.tile_pool(name="tmp", bufs=4))
        tsub = ctx.enter_context(tc.tile_pool(name="tsub", bufs=5))
        mbrb = ctx.enter_context(tc.tile_pool(name="mbrb", bufs=2))
        srows = ctx.enter_context(tc.tile_pool(name="srows", bufs=3))
        srows1 = ctx.enter_context(tc.tile_pool(name="srows1", bufs=2))
        rows = ctx.enter_context(tc.tile_pool(name="rows", bufs=2))
        params = ctx.enter_context(tc.tile_pool(name="params", bufs=2))
        # PSUM budget: 8 banks = pscore 2x2 + pmm 2x1 + patt 2x1
        pscore = ctx.enter_context(tc.tile_pool(name="pscore", bufs=2, space="PSUM"))
        pmm = ctx.enter_context(tc.tile_pool(name="pmm", bufs=2, space="PSUM"))
        patt = ctx.enter_context(tc.tile_pool(name="patt", bufs=2, space="PSUM"))

        # ---------------- constants ----------------
        xT = state.tile([P, KT, S], bf16, tag="xT")
        ones_f32 = consts.tile([P, 1], f32, tag="ones_f32")
        nc.vector.memset(ones_f32[:, :], 1.0)
        ones_rf32 = consts.tile([1, QW], f32, tag="ones_rf32")
        nc.vector.memset(ones_rf32[:, :], 1.0)
        ones_col = consts.tile([P, 1], bf16, tag="ones_col")
        nc.vector.tensor_copy(out=ones_col[:, :], in_=ones_f32[:, :])
        ones_colr = consts.tile([P, 1], f32r, tag="ones_colr")
        nc.vector.tensor_copy(out=ones_colr[:, :], in_=ones_f32[:, :])
        ones_row = consts.tile([1, QW], f32r, tag="ones_row")
        nc.vector.tensor_copy(out=ones_row[:, :], in_=ones_rf32[:, :])
        ones_rbf = consts.tile([1, QW], bf16, tag="ones_rbf")
        nc.vector.tensor_copy(out=ones_rbf[:, :], in_=ones_rf32[:, :])
        ident = consts.tile([P, P], bf16, tag="ident")
        make_identity(nc, ident[:, :])
        eps_sb = consts.tile([1, 1], f32, tag="eps")
        nc.vector.memset(eps_sb[:, :], EPS)
        ids_sb = consts.tile([P, NT], i32, tag="ids")
        nc.sync.dma_start(out=ids_sb[:, :], in_=d_ids[:, :])
        gcol = consts.tile([P, NT], f32, tag="gcol")
        nc.sync.dma_start(out=gcol[:, :], in_=d_gm[:, :])
        gcol_s = consts.tile([P, NT], f32, tag="gcol_s")
        nc.vector.tensor_scalar(
            out=gcol_s[:, :], in0=gcol[:, :],
            scalar1=(AV_SCALE / FP8_SCALE if FP8_QKV else AV_SCALE), scalar2=None,
            op0=OP.mult,
        )
        gcol_m = consts.tile([P, NT], f32, tag="gcol_m")
        nc.vector.tensor_scalar(
            out=gcol_m[:, :], in0=gcol[:, :],
            scalar1=AV_SCALE, scalar2=None, op0=OP.mult,
        )
        cw_sb = consts.tile([P, KT, C], f32r, tag="cw")
        nc.sync.dma_start(out=cw_sb[:, :, :], in_=d_cw[:, :, :].bitcast(f32r))
        cb_sb = consts.tile([1, C], f32r, tag="cb")
        nc.sync.dma_start(out=cb_sb[:, :], in_=d_cb[:, :].bitcast(f32r))

        # persistent v tile: right half holds the key-padding mask column
        # replicated 64x (written once; av matmuls then leave the softmax
        # denominator replicated in psum rows 64:128)
        v_sb = vpool.tile([P, NT, H, 2 * HD], vdt, tag="v")
        for t in range(NT):
            nc.vector.tensor_copy(
                out=v_sb[:, t, :, HD : 2 * HD],
                in_=gcol_m[:, t : t + 1].to_broadcast([P, H, HD]),
            )

        # per-layer weight prefetch (one slot ahead via bufs=2 rings)
        layer_w = {}

        def issue_layer_weights(l):
            qkw_sb = qkwp.tile([P, 12, KT, P], wdt, tag="qkw", name="qkw_sb")
            nc.sync.dma_start(out=qkw_sb[:, :, :, :], in_=d_qkw[l])
            vw_sb = qkwp.tile([P, KT, D], wdt, tag="vw", name="vw_sb")
            nc.sync.dma_start(out=vw_sb[:, :, :], in_=d_vw[l])
            vb_row = rows.tile([1, D], bf16, tag="brow")
            nc.sync.dma_start(out=vb_row[:, :], in_=d_vb[l : l + 1, :])
            bias_sb = params.tile([P, NBIAS], f32, tag="bias")
            nc.sync.dma_start(out=bias_sb[:, :], in_=d_bias[l])
            layer_w[l] = (qkw_sb, vw_sb, vb_row, bias_sb)

        issue_layer_weights(0)

        # ---------------- embedding ----------------
        posT_sb = b24.tile([P, KT, S], bf16, tag="b24")
        nc.sync.dma_start(
            out=posT_sb[:, :, :], in_=d_posT.rearrange("(j p) s -> p j s", p=P)
        )
        embts = {}

        def emb_gather(t):
            embt = tmp.tile([P, D], bf16, tag="tmp")
            nc.gpsimd.indirect_dma_start(
                out=embt[:, :],
                out_offset=None,
                in_=d_emb[:, :],
                in_offset=bass.IndirectOffsetOnAxis(ap=ids_sb[:, t : t + 1], axis=0),
            )
            embts[t] = embt

        # 3-deep prefetch: never allocate a ring slot before its previous
        # occupant's readers have been emitted
        for t in range(3):
            emb_gather(t)
        for t in range(NT):
            if t + 3 < NT:
                emb_gather(t + 3)
            embt = embts.pop(t)
            tr = pmm.tile([P, KT, P], bf16, tag="pmm")
            for j in range(KT):
                nc.tensor.transpose(
                    out=tr[:, j, :],
                    in_=embt[:, j * P : (j + 1) * P],
                    identity=ident[:, :],
                )
            nc.vector.tensor_tensor(
                out=xT[:, :, ts(t, P)], in0=tr[:, :, :],
                in1=posT_sb[:, :, ts(t, P)], op=OP.add,
            )

        # ---------------- layer norm (split into stats / finish) ----------
        def ln_stats_start():
            st = pscore.tile([1, 2 * QW], f32, tag="ps", name="st")
            return st

        def ln_stats_step(st, src, n, j):
            nsl = ds(n * QW, QW)
            sq = tmp.tile([P, QW], f32r, tag="tmp")
            nc.scalar.square(sq[:, :], src[:, j, nsl])
            nc.tensor.matmul(
                st[:, 0:QW], ones_col[:, :], src[:, j, nsl],
                start=(j == 0), stop=(j == KT - 1),
            )
            nc.tensor.matmul(
                st[:, QW : 2 * QW], ones_colr[:, :], sq[:, :],
                start=(j == 0), stop=(j == KT - 1),
            )

        def ln_rows(st):
            """Mean/rstd row chain for one query half."""
            mean = srows1.tile([1, QW], f32r, tag="mean")
            nc.vector.tensor_scalar(
                out=mean[:, :], in0=st[:, 0:QW], scalar1=1.0 / D, scalar2=None,
                op0=OP.mult,
            )
            msq = srows.tile([1, QW], f32, tag="srow")
            nc.vector.tensor_scalar(
                out=msq[:, :], in0=st[:, QW : 2 * QW], scalar1=1.0 / D,
                scalar2=None, op0=OP.mult,
            )
            var = srows.tile([1, QW], f32, tag="srow")
            nc.vector.scalar_tensor_tensor(
                out=var[:, :], in0=mean[:, :], scalar=-1.0, in1=mean[:, :],
                op0=OP.mult, op1=OP.mult,
            )
            nc.vector.tensor_tensor(
                out=var[:, :], in0=var[:, :], in1=msq[:, :], op=OP.add,
            )
            lnv = srows.tile([1, QW], f32, tag="srow")
            nc.scalar.activation(lnv[:, :], var[:, :], AF.Ln, bias=eps_sb[:, :])
            rstd = srows1.tile([1, QW], f32r, tag="rstd")
            nc.scalar.activation(rstd[:, :], lnv[:, :], AF.Exp, scale=-0.5)
            return mean, rstd

        def ln_apply(mean_rstd, src, dst, n):
            """Broadcast mean/rstd across partitions and normalize."""
            mean, rstd = mean_rstd
            nsl = ds(n * QW, QW)
            bc = pscore.tile([P, 2, QW], f32, tag="ps", name="bc")
            nc.tensor.matmul(
                bc[:, 0, :], ones_row[0:1, 0:P], mean[:, :],
                start=True, stop=True,
            )
            nc.tensor.matmul(
                bc[:, 1, :], ones_row[0:1, 0:P], rstd[:, :],
                start=True, stop=True,
            )
            mr = mbrb.tile([P, 2, QW], bf16, tag="mbrb")
            nc.scalar.copy(out=mr[:, :, :], in_=bc[:, :, :])
            for j in range(KT):
                t1 = tsub.tile([P, QW], bf16, tag="tsub")
                nc.vector.tensor_tensor(
                    out=t1[:, :], in0=src[:, j, nsl], in1=mr[:, 0, :],
                    op=OP.subtract,
                )
                nc.vector.tensor_tensor(
                    out=dst[:, j, nsl], in0=t1[:, :], in1=mr[:, 1, :],
                    op=OP.mult,
                )

        # ---------------- layers ----------------
        ln1_st = {}
        for n in range(NQ):
            st = ln_stats_start()
            for j in range(KT):
                ln_stats_step(st, xT, n, j)
            ln1_st[n] = st

        ln1_rows = {}
        hT_next = None
        for l in range(n_layers):
            qkw_sb, vw_sb, vb_row, bias_sb = layer_w.pop(l)
            hT = hT_next if hT_next is not None else b24.tile(
                [P, KT, S], wdt, tag="b24", name="hT")
            hT_next = None
            qk_sb = p48.tile([P, 12, S], bf16, tag="p48")
            attT = attp.tile([P, KT, S], bf16, tag="attT")

            def v_proj_half(t, c0, cn):
                for c0, cn in ((c0, cn),):
                    acc = pmm.tile([P, QW], f32, tag="pmm", name="vacc")
                    nc.tensor.matmul(
                        acc[:, 0:cn], ones_rbf[0:1, 0:P], vb_row[:, c0 : c0 + cn],
                        start=True, stop=False,
                    )
                    if FP8_QKV:
                        for i in range(KT // 2):
                            nc.tensor.matmul(
                                acc[:, 0:cn],
                                hT[:, 2 * i : 2 * i + 2, ts(t, P)],
                                vw_sb[:, 2 * i : 2 * i + 2, c0 : c0 + cn],
                                start=False, stop=(i == KT // 2 - 1),
                                perf_mode=DR,
                            )
                    else:
                        for j in range(KT):
                            nc.tensor.matmul(
                                acc[:, 0:cn], hT[:, j, ts(t, P)],
                                vw_sb[:, j, c0 : c0 + cn],
                                start=False, stop=(j == KT - 1),
                            )
                    nc.vector.tensor_scalar(
                        out=v_sb[:, t, c0 // HD : (c0 + cn) // HD, 0:HD],
                        in0=acc[:, 0:cn].rearrange("p (h d) -> p h d", d=HD),
                        scalar1=gcol_s[:, t : t + 1], scalar2=None, op0=OP.mult,
                    )

            def qk_group(m, n):
                nsl = ds(n * QW, QW)
                acc = pmm.tile([P, QW], f32, tag="pmm", name="qkacc")
                if FP8_QKV:
                    for i in range(KT // 2):
                        nc.tensor.matmul(
                            acc[:, :],
                            qkw_sb[:, m, 2 * i : 2 * i + 2, :],
                            hT[:, 2 * i : 2 * i + 2, nsl],
                            start=(i == 0), stop=(i == KT // 2 - 1),
                            perf_mode=DR,
                        )
                    nc.vector.tensor_scalar(
                        out=qk_sb[:, m, nsl], in0=acc[:, :],
                        scalar1=1.0 / FP8_SCALE,
                        scalar2=bias_sb[:, BQK + m : BQK + m + 1],
                        op0=OP.mult, op1=OP.add,
                    )
                else:
                    for j in range(KT):
                        nc.tensor.matmul(
                            acc[:, :], qkw_sb[:, m, j, :], hT[:, j, nsl],
                            start=(j == 0), stop=(j == KT - 1),
                        )
                    nc.vector.tensor_scalar(
                        out=qk_sb[:, m, nsl], in0=acc[:, :],
                        scalar1=bias_sb[:, BQK + m : BQK + m + 1], scalar2=None,
                        op0=OP.add,
                    )

            # LN1 finish + minimal serial prefix: only pair-0 q/k and the
            # first two v tiles run before the score streams; the remaining
            # v tiles and later pairs' q/k ride inside the streams (the PE
            # has slack there — the streams are ACT-exp-bound).
            def v_proj_t(t):
                v_proj_half(t, 0, QW)
                v_proj_half(t, QW, D - QW)

            if 0 in ln1_st:
                ln1_rows[0] = ln_rows(ln1_st.pop(0))
            if 0 in ln1_rows:
                ln_apply(ln1_rows.pop(0), xT, hT, 0)
            # n=0-half work first: hT n=0 is ready from the previous layer's
            # tail, so PE starts immediately; the n=1 row chain + normalize
            # run on ACT/DVE behind it
            v_proj_t(0)
            v_proj_t(1)
            qk_group(6, 0)
            qk_group(0, 0)
            if 1 in ln1_st:
                ln1_rows[1] = ln_rows(ln1_st.pop(1))
            if 1 in ln1_rows:
                ln_apply(ln1_rows.pop(1), xT, hT, 1)
            v_proj_t(2)
            v_proj_t(3)
            qk_group(6, 1)
            qk_group(0, 1)
            vq_queue = []
            for t in range(4, NT):
                vq_queue.append(("v", t, 0, QW))
                vq_queue.append(("v", t, QW, D - QW))

            # out-proj weights prefetched behind the attention stream
            ow_sb = vwpool.tile([P, KT, KT, P], bf16, tag="ow")
            nc.sync.dma_start(out=ow_sb[:, :, :, :], in_=d_ow[l])

            ln2_st = {}

            def out_proj_half(n, with_stats):
                nsl = ds(n * QW, QW)
                st2 = ln_stats_start() if with_stats else None
                for m in range(KT):
                    acc = pmm.tile([P, QW], f32, tag="pmm", name="oacc")
                    for j in range(KT):
                        nc.tensor.matmul(
                            acc[:, :], ow_sb[:, m, j, :], attT[:, j, nsl],
                            start=(j == 0), stop=(j == KT - 1),
                        )
                    nc.vector.scalar_tensor_tensor(
                        out=xT[:, m, nsl], in0=acc[:, :],
                        scalar=bias_sb[:, BOB + m : BOB + m + 1],
                        in1=xT[:, m, nsl],
                        op0=OP.add, op1=OP.add,
                    )
                    if with_stats:
                        ln_stats_step(st2, xT, n, m)
                if with_stats:
                    ln2_st[n] = st2

            for hp in range(H // 2):
                hA, hB = 2 * hp, 2 * hp + 1
                if hp + 1 < H // 2:
                    # k groups first: scores kt spans both halves of k
                    vq_queue += [("qk", 7 + hp, 0), ("qk", 7 + hp, 1),
                                 ("qk", 1 + hp, 0), ("qk", 1 + hp, 1)]
                pend = deque()
                pats = {}

                if AV_FP8:
                    def av_emit(n, ktp, epair):
                        if ktp == 0:
                            patA = patt.tile([P, QW], f32, tag="patt", name="patA")
                            patB = patt.tile([P, QW], f32, tag="patt", name="patB")
                            pats[n] = (patA, patB)
                        pA, pB = pats[n]
                        nc.tensor.matmul(
                            pA[:, :], v_sb[:, 2 * ktp : 2 * ktp + 2, hA, :],
                            epair[:, :, 0:QW],
                            start=(ktp == 0), stop=(ktp == NT // 2 - 1),
                            perf_mode=DR,
                        )
                        nc.tensor.matmul(
                            pB[:, :], v_sb[:, 2 * ktp : 2 * ktp + 2, hB, :],
                            epair[:, :, QW : 2 * QW],
                            start=(ktp == 0), stop=(ktp == NT // 2 - 1),
                            perf_mode=DR,
                        )
                else:
                    def av_emit(n, kt, e):
                        if kt == 0:
                            patA = patt.tile([P, QW], f32, tag="patt", name="patA")
                            patB = patt.tile([P, QW], f32, tag="patt", name="patB")
                            pats[n] = (patA, patB)
                        pA, pB = pats[n]
                        nc.tensor.matmul(
                            pA[:, :], v_sb[:, kt, hA, :], e[:, 0:QW],
                            start=(kt == 0), stop=(kt == NT - 1),
                        )
                        nc.tensor.matmul(
                            pB[:, :], v_sb[:, kt, hB, :], e[:, QW : 2 * QW],
                            start=(kt == 0), stop=(kt == NT - 1),
                        )

                for n in range(NQ):
                    nsl = ds(n * QW, QW)
                    epair = None
                    for kt in range(NT):
                        ps = pscore.tile([P, 2 * QW], f32, tag="ps")
                        nc.tensor.matmul(
                            ps[:, 0:QW],
                            qk_sb[0:HD, 6 + hp, ts(kt, P)],
                            qk_sb[0:HD, hp, nsl],
                            start=True, stop=True,
                        )
                        nc.tensor.matmul(
                            ps[:, QW : 2 * QW],
                            qk_sb[HD:P, 6 + hp, ts(kt, P)],
                            qk_sb[HD:P, hp, nsl],
                            start=True, stop=True,
                        )
                        if AV_FP8:
                            if kt % 2 == 0:
                                epair = epool.tile([P, 2, 2 * QW], f8, tag="e",
                                                   name="epair")
                            if kt in SCHRAUD_KTS:
                                nc.vector.tensor_scalar(
                                    out=epair[:, kt % 2, :].bitcast(u8),
                                    in0=ps[:, :], scalar1=SCHRAUD_C1,
                                    scalar2=SCHRAUD_C2,
                                    op0=OP.mult, op1=OP.add,
                                )
                            else:
                                nc.scalar.activation(
                                    epair[:, kt % 2, :], ps[:, :], AF.Exp)
                            if kt % 2 == 1:
                                pend.append((n, kt // 2, epair))
                                if len(pend) >= 2:
                                    av_emit(*pend.popleft())
                        else:
                            e = epool.tile([P, 2 * QW], bf16, tag="e")
                            nc.scalar.activation(e[:, :], ps[:, :], AF.Exp)
                            pend.append((n, kt, e))
                            if len(pend) >= 3:
                                av_emit(*pend.popleft())
                        if 1 <= kt <= 6 and vq_queue:
                            # two pops when both are v-halves: the first
                            # stream must emit every v write before the av
                            # that reads it (program order = dependency
                            # order for the tile framework)
                            for _ in range(2 if vq_queue[0][0] == "v" else 1):
                                if not vq_queue:
                                    break
                                task = vq_queue.pop(0)
                                if task[0] == "v":
                                    v_proj_half(task[1], task[2], task[3])
                                else:
                                    qk_group(task[1], task[2])
                    while pend:
                        av_emit(*pend.popleft())
                    # drain: denominator is replicated in psum rows 64:128
                    pA, pB = pats.pop(n)
                    for pat, po in ((pA, 0), (pB, HD)):
                        zinv = srows.tile([HD, QW], bf16, tag="zinv")
                        with nc.allow_low_precision(reason="softmax denom bf16"):
                            nc.vector.reciprocal(zinv[:, :], pat[HD:P, :])
                        nc.vector.tensor_tensor(
                            out=attT[po : po + HD, hp, nsl],
                            in0=pat[0:HD, :], in1=zinv[:, :], op=OP.mult,
                        )
                    if hp == H // 2 - 1 and n == 0:
                        # n=0 attention fully drained: overlap the n=0
                        # out-projection with the last pair's n=1 stream
                        out_proj_half(0, with_stats=False)
                while vq_queue:
                    task = vq_queue.pop(0)
                    if task[0] == "v":
                        v_proj_half(task[1], task[2], task[3])
                    else:
                        qk_group(task[1], task[2])

            # ---- output projection + residual, n-split, LN2 stats behind;
            # the n=0 half was emitted inside the last head pair's n=1
            # attention stream ----
            st2 = ln_stats_start()
            for m in range(KT):
                ln_stats_step(st2, xT, 0, m)
            ln2_st[0] = st2
            # n=0 row chain + normalize issued before out_proj(1) so all
            # their ACT/DVE latency hides behind those matmuls and ff1 can
            # start the moment out_proj(1) drains
            r0 = ln_rows(ln2_st.pop(0))
            h2 = b24.tile([P, KT, S], bf16, tag="b24")
            ln_apply(r0, xT, h2, 0)
            out_proj_half(1, with_stats=True)
            r1 = ln_rows(ln2_st.pop(1))
            ln_apply(r1, xT, h2, 1)

            f1w_tiles = {}

            def dma_f1w(ch):
                wt = w6.tile([P, FCM, KT, P], bf16, tag="w6", name="f1wc")
                nc.sync.dma_start(out=wt[:, :, :, :], in_=d_f1w[l, ch])
                f1w_tiles[ch] = wt

            f2w_tiles = {}

            def dma_f2w(m):
                w2 = wff2.tile([P, FT, P], bf16, tag="wff2", name="f2wt")
                nc.sync.dma_start(out=w2[:, :, :], in_=d_f2w[l, m])
                f2w_tiles[m] = w2

            dma_f1w(0)
            dma_f1w(1)
            if l + 1 < n_layers:
                issue_layer_weights(l + 1)
            dma_f2w(0)
            dma_f2w(1)

            f_sb = p48.tile([P, FT, S], bf16, tag="p48")
            for ch in range(FCH):
                if ch >= 1 and ch + 1 < FCH:
                    dma_f1w(ch + 1)
                wt = f1w_tiles.pop(ch)
                # n-major within the chunk: the first chunk's n=0 groups can
                # start as soon as the n=0 normalize lands, hiding the n=1
                # apply latency behind them
                for n in range(NQ):
                    for mi in range(FCM):
                        m = ch * FCM + mi
                        nsl = ds(n * QW, QW)
                        acc = pmm.tile([P, QW], f32, tag="pmm", name="facc")
                        for j in range(KT):
                            nc.tensor.matmul(
                                acc[:, :], wt[:, mi, j, :], h2[:, j, nsl],
                                start=(j == 0), stop=(j == KT - 1),
                            )
                        # drain on ACT (idle during FFN; DVE is the busy one)
                        nc.scalar.activation(
                            f_sb[:, m, nsl], acc[:, :], AF.Relu,
                            bias=bias_sb[:, BF1 + m : BF1 + m + 1],
                        )

            last = l == n_layers - 1
            st1 = {} if last else {n: ln_stats_start() for n in range(NQ)}

            def ff2_half(m, n):
                nsl = ds(n * QW, QW)
                w2 = f2w_tiles[m]
                acc = pmm.tile([P, QW], f32, tag="pmm", name="f2acc")
                for j in range(FT):
                    nc.tensor.matmul(
                        acc[:, :], w2[:, j, :], f_sb[:, j, nsl],
                        start=(j == 0), stop=(j == FT - 1),
                    )
                nc.vector.scalar_tensor_tensor(
                    out=xT[:, m, nsl], in0=acc[:, :],
                    scalar=bias_sb[:, BF2 + m : BF2 + m + 1],
                    in1=xT[:, m, nsl],
                    op0=OP.add, op1=OP.add,
                )
                if not last:
                    ln_stats_step(st1[n], xT, n, m)

            # m-pairs with n=0 first within each pair: st1[0] closes two
            # groups before the end, so the next layer's first-half
            # normalize hides behind the final n=1 groups
            order = [(0, 0), (1, 0), (0, 1), (1, 1), (2, 0), (3, 0),
                     (2, 1), (3, 1), (4, 0), (5, 0), (4, 1), (5, 1)]
            done_n = {m: 0 for m in range(KT)}
            for m, n in order:
                ff2_half(m, n)
                done_n[m] += 1
                if done_n[m] == NQ:
                    f2w_tiles.pop(m)
                    if m + 2 < KT:
                        dma_f2w(m + 2)
                if (m, n) == (KT - 1, 0) and not last:
                    ln1_rows[0] = ln_rows(st1.pop(0))
                    hT_next = b24.tile([P, KT, S], wdt, tag="b24", name="hTn")
                    ln_apply(ln1_rows.pop(0), xT, hT_next, 0)
            if not last:
                ln1_st[1] = st1[1]

        # ---------------- CLS head ----------------
        col2 = xT[:, :, 0:2]  # (P, KT, 2) bf16
        xsqc = consts.tile([P, KT, 2], bf16, tag="xsqc")
        nc.scalar.square(xsqc[:, :, :], col2)
        pss = pmm.tile([1, QW], f32, tag="pmm")
        for j in range(KT):
            nc.tensor.matmul(
                pss[:, 0:2], ones_col[:, :], xT[:, j, 0:2],
                start=(j == 0), stop=(j == KT - 1),
            )
        for j in range(KT):
            nc.tensor.matmul(
                pss[:, 2:4], ones_col[:, :], xsqc[:, j, :],
                start=(j == 0), stop=(j == KT - 1),
            )
        hmean = srows1.tile([1, 64], f32r, tag="mean")
        nc.vector.tensor_scalar(
            out=hmean[:, 0:2], in0=pss[:, 0:2], scalar1=1.0 / D, scalar2=None,
            op0=OP.mult,
        )
        hmsq = srows.tile([1, 64], f32, tag="srow")
        nc.vector.tensor_scalar(
            out=hmsq[:, 0:2], in0=pss[:, 2:4], scalar1=1.0 / D, scalar2=None,
            op0=OP.mult,
        )
        hvar = srows.tile([1, 64], f32, tag="srow")
        nc.vector.scalar_tensor_tensor(
            out=hvar[:, 0:2], in0=hmean[:, 0:2], scalar=-1.0, in1=hmean[:, 0:2],
            op0=OP.mult, op1=OP.mult,
        )
        nc.vector.tensor_tensor(
            out=hvar[:, 0:2], in0=hvar[:, 0:2], in1=hmsq[:, 0:2], op=OP.add
        )
        hlnv = srows.tile([1, 64], f32, tag="srow")
        nc.scalar.activation(hlnv[:, 0:2], hvar[:, 0:2], AF.Ln, bias=eps_sb[:, :])
        hrstd = srows1.tile([1, 64], f32r, tag="rstd")
        nc.scalar.activation(hrstd[:, 0:2], hlnv[:, 0:2], AF.Exp, scale=-0.5)
        pbc = pmm.tile([P, QW], f32, tag="pmm")
        nc.tensor.matmul(pbc[:, 0:2], ones_row[0:1, 0:P], hmean[:, 0:2],
                         start=True, stop=True)
        nc.tensor.matmul(pbc[:, 2:4], ones_row[0:1, 0:P], hrstd[:, 0:2],
                         start=True, stop=True)
        t1 = consts.tile([P, KT, 2], f32, tag="ht1")
        nc.vector.tensor_tensor(
            out=t1[:, :, :], in0=col2, in1=pbc[:, 0:1].to_broadcast([P, KT, 2]),
            op=OP.subtract,
        )
        pc = consts.tile([P, KT, 2], f32r, tag="pc")
        nc.vector.tensor_tensor(
            out=pc[:, :, :], in0=t1[:, :, :], in1=pbc[:, 2:3].to_broadcast([P, KT, 2]),
            op=OP.mult,
        )
        plog = patt.tile([P, QW], f32, tag="patt")
        nc.tensor.matmul(
            plog[0:C, 0:2], cb_sb[:, :], ones_row[:, 0:2], start=True, stop=False
        )
        for j in range(KT):
            nc.tensor.matmul(
                plog[0:C, 0:2], cw_sb[:, j, :], pc[:, j, :],
                start=False, stop=(j == KT - 1),
            )
        out_sb = consts.tile([C, 1], f32, tag="outsb")
        nc.vector.tensor_copy(out=out_sb[:, :], in_=plog[0:C, 0:1])
        nc.sync.dma_start(out=d_out[0:1, 0:C], in_=out_sb[0:C, 0:1])

    nc.compile()
    _patch_act_tables(nc)
    return nc


def _patch_act_tables(nc):
    """All activation functions used here (exp, ln, square, copy, relu,
    identity) live together in act func set 6 (natural_log_exp_and_others),
    but the table-load pass picks the first set containing each function,
    inserting ~50 1.3us reloads.  Retarget the first load to set 6 and drop
    the redundant ones (they carry no semaphore info)."""
    from concourse import mybir

    first = True
    for b in nc.m.functions[0].blocks:
        keep = []
        for inst in b.instructions:
            if isinstance(inst, mybir.InstLoadActFuncSet):
                si = inst.sync_info
                has_sems = si is not None and (
                    len(si.on_wait) > 0 or len(si.on_update) > 0
                )
                if first or has_sems:
                    inst.act_func_set_id = 6
                    keep.append(inst)
                    first = False
            else:
                keep.append(inst)
        b.instructions[:] = keep


def _bf16np():
    import ml_dtypes

    return ml_dtypes.bfloat16


def _f8np():
    import ml_dtypes

    return ml_dtypes.float8_e4m3fn


def _prep_host(inputs):
    g = lambda k: np.asarray(inputs[k])
    bf = _bf16np()
    sq = np.float32(math.sqrt(D))
    ids = g("input_ids").astype(np.int32)              # (B, S)
    gm = (1.0 - g("attention_mask").astype(np.float32))  # (B, S)
    emb = (g("token_emb").astype(np.float32) * sq)
    posT = np.ascontiguousarray((g("pos_emb")[:S].astype(np.float32) * sq).T)
    # reference reshapes qkv output to (H, 3, HD): permute columns into
    # contiguous q | k | v blocks (each h-major) before tiling
    idx = np.arange(3 * D).reshape(H, 3, HD)
    cols = np.concatenate(
        [idx[:, 0, :].reshape(-1), idx[:, 1, :].reshape(-1), idx[:, 2, :].reshape(-1)]
    )
    qkv_w_orig = g("qkv_w").astype(np.float32)[:, :, cols].copy()  # (L, D, 3D)
    qkv_b = g("qkv_b").astype(np.float32)[:, cols].copy()          # (L, 3D)
    qkv_w_orig[:, :, :D] *= np.float32(1.0 / math.sqrt(HD))
    qkv_b[:, :D] *= np.float32(1.0 / math.sqrt(HD))
    # fold LN1 scale/bias into qkv:  (x*s+b) @ W = x @ (diag(s)W) + (b@W)
    n1_s = g("n1_s").astype(np.float32)   # (L, D)
    n1_b = g("n1_b").astype(np.float32)
    qkv_w = qkv_w_orig * n1_s[:, :, None]
    qkv_b = qkv_b + np.einsum("ld,lde->le", n1_b, qkv_w_orig)
    # fold LN2 into ff1
    n2_s = g("n2_s").astype(np.float32)
    n2_b = g("n2_b").astype(np.float32)
    ff1_w_orig = g("ff1_w").astype(np.float32)          # (L, D, DF)
    ff1_w = ff1_w_orig * n2_s[:, :, None]
    ff1_b = g("ff1_b").astype(np.float32) + np.einsum("ld,ldf->lf", n2_b, ff1_w_orig)
    # fold head LN into cls
    hln_s = g("hln_s").astype(np.float32)
    hln_b = g("hln_b").astype(np.float32)
    cls_w_orig = g("cls_w").astype(np.float32)          # (D, C)
    cls_w = cls_w_orig * hln_s[:, None]
    cls_b = g("cls_b").astype(np.float32) + hln_b @ cls_w_orig

    wnp = _f8np() if FP8_QKV else bf
    qw_scale = np.float32(FP8_SCALE) if FP8_QKV else np.float32(1.0)
    # packed per-layer biases: [qkb(12) | ob(6) | f1b(24) | f2b(6)] as [L,P,48]
    qkb_t = qkv_b[:, : 2 * D].reshape(L, 12, P).transpose(0, 2, 1)
    ob_t = g("out_b").astype(np.float32).reshape(L, KT, P).transpose(0, 2, 1)
    f1b_t = ff1_b.reshape(L, FT, P).transpose(0, 2, 1)
    f2b_t = g("ff2_b").astype(np.float32).reshape(L, KT, P).transpose(0, 2, 1)
    biases = np.ascontiguousarray(
        np.concatenate([qkb_t, ob_t, f1b_t, f2b_t], axis=2)
    )
    shared = {
        "emb": emb.astype(bf),
        "posT": posT.astype(bf),
        "qkw": np.ascontiguousarray(
            (qkv_w[:, :, : 2 * D] * qw_scale)
            .reshape(L, KT, P, 12, P).transpose(0, 2, 3, 1, 4)
        ).astype(wnp),
        "vw": np.ascontiguousarray(
            (qkv_w[:, :, 2 * D :] * qw_scale)
            .reshape(L, KT, P, D).transpose(0, 2, 1, 3)
        ).astype(wnp),
        "vb": np.ascontiguousarray(qkv_b[:, 2 * D :] * qw_scale).astype(bf),
        "biases": biases,
        "ow": np.ascontiguousarray(
            g("out_w").astype(np.float32).reshape(L, KT, P, KT, P).transpose(0, 2, 3, 1, 4)
        ).astype(bf),
        "f1w": np.ascontiguousarray(
            ff1_w.reshape(L, KT, P, FT, P).transpose(0, 3, 2, 1, 4)
            .reshape(L, FCH, FCM, P, KT, P).transpose(0, 1, 3, 2, 4, 5)
        ).astype(bf),
        "f2w": np.ascontiguousarray(
            g("ff2_w").astype(np.float32).reshape(L, FT, P, KT, P).transpose(0, 3, 2, 1, 4)
        ).astype(bf),
        "cw": np.ascontiguousarray(cls_w.reshape(KT, P, C).transpose(1, 0, 2)),
        "cb": cls_b.reshape(1, C),
    }
    per_core = []
    for c in range(N_CORES):
        per_core.append(
            {
                "ids": np.ascontiguousarray(ids[c].reshape(NT, P).T),
                "gmask": np.ascontiguousarray(gm[c].reshape(NT, P).T),
            }
        )
    return shared, per_core


def _get_nc():
    if "nc" not in _CACHE:
        _CACHE["nc"] = _build_nc()
    return _CACHE["nc"]


def kernel(**inputs):
    from concourse.bass_utils import run_bass_kernel_spmd

    shared, per_core = _prep_host(inputs)
    nc = _get_nc()
    in_maps = [dict(shared, **per_core[c]) for c in range(N_CORES)]
    _CACHE["in_maps"] = in_maps
    res = run_bass_kernel_spmd(nc, in_maps, list(range(N_CORES)))
    out = np.stack([res.results[c]["out"][0] for c in range(N_CORES)], axis=0)
    return out.astype(np.float32)


def bench(n_iters=10):
    """Re-run the compiled NEFF with device-resident inputs; returns the
    best-observed per-iteration wall time in ns (upper bound on HW exec)."""
    import time

    import jax
    import numpy as _np
    from jax.sharding import Mesh, PartitionSpec, NamedSharding
    from jax.experimental.shard_map import shard_map
    from concourse import bass2jax, mybir
    from concourse.bass2jax import _bass_exec_p, install_neuronx_cc_hook

    nc = _get_nc()
    in_maps = _CACHE["in_maps"]
    install_neuronx_cc_hook()

    pname = nc.partition_id_tensor.name if nc.partition_id_tensor else None
    in_names, out_names, out_avals, zero_outs = [], [], [], []
    for alloc in nc.m.functions[0].allocations:
        if not isinstance(alloc, mybir.MemoryLocationSet):
            continue
        name = alloc.memorylocations[0].name
        if alloc.kind == "ExternalInput":
            if name == pname:
                continue
            in_names.append(name)
        elif alloc.kind == "ExternalOutput":
            out_names.append(name)
            shape = tuple(alloc.tensor_shape)
            dtype = mybir.dt.np(alloc.dtype)
            out_avals.append(jax.core.ShapedArray(shape, dtype))
            zero_outs.append(_np.zeros(shape, dtype))
    n_params = len(in_names)
    all_names = in_names + out_names
    if pname is not None:
        all_names = all_names + [pname]

    def _body(*args):
        operands = list(args)
        if pname is not None:
            operands.append(bass2jax.partition_id_tensor())
        outs = _bass_exec_p.bind(
            *operands,
            out_avals=tuple(out_avals),
            in_names=tuple(all_names),
            out_names=tuple(out_names),
            lowering_input_output_aliases=(),
            sim_require_finite=True,
            sim_require_nnan=True,
            nc=nc,
        )
        return tuple(outs)

    devices = jax.devices()[:N_CORES]
    mesh = Mesh(_np.asarray(devices), ("core",))
    nin = n_params + len(zero_outs)
    fn = jax.jit(
        shard_map(
            _body,
            mesh=mesh,
            in_specs=(PartitionSpec("core"),) * nin,
            out_specs=(PartitionSpec("core"),) * len(out_names),
            check_rep=False,
        )
    )
    sharding = NamedSharding(mesh, PartitionSpec("core"))
    concat_in = [
        jax.device_put(
            _np.concatenate([_np.asarray(in_maps[c][n]) for c in range(N_CORES)], 0),
            sharding,
        )
        for n in in_names
    ]
    concat_zeros = [
        jax.device_put(
            _np.zeros((N_CORES * z.shape[0], *z.shape[1:]), z.dtype), sharding
        )
        for z in zero_outs
    ]
    jax.block_until_ready(concat_in)
    # warmup (compile)
    out = fn(*concat_in, *concat_zeros)
    jax.block_until_ready(out)
    # pipelined async dispatch amortizes the axon tunnel round-trip
    outs = []
    t0 = time.perf_counter()
    for _ in range(n_iters):
        outs.append(fn(*concat_in, *concat_zeros))
    jax.block_until_ready(outs)
    dt = (time.perf_counter() - t0) / n_iters
    return int(dt * 1e9)


# revision 54
# speedup vs baseline: 1.8984x; 1.1712x over previous
"""Bot-detection transformer forward pass on 8 Trainium2 NeuronCores.

Strategy: data-parallel over batch (B=8 -> 1 sequence per core, no
collectives).  Residual stream kept transposed (feature-major,
xT: [768 x 1024] as 6 partition-tiles) in bf16 so projections run with
weights stationary / activations moving (full PE rate) and DVE element
ops hit the 2x 16-bit mode.

The q/k/v projections run in fp8e4 DoubleRow mode (2 contraction tiles
per matmul at 0.5 cycles/row): hT is produced in fp8e4 by the LN
normalize, the weights are scaled x32 on the host to sit in e4m3's
normal range, and the 1/32 rides the existing drain tensor_scalar ops.
The att.v matmuls also run fp8 DoubleRow (v and exp(scores) in fp8,
two key tiles contracted per pass); scores and the FFN stay bf16
(fp8 there fails the 2e-2 gate).

LayerNorm scale/bias are folded into the following projection weights
on the host, so the on-device LN is only (x-mean)*rstd; stats (sums +
sums of ACT-engine squares) ride ones-matmuls interleaved behind the
previous stage's drains, the mean/rstd row chain runs at the consumer,
and the next layer's first-half normalize is pre-computed behind the
tail of ff2 so the layer boundary starts with hT ready.  All activation
functions used live in one table (natural_log_exp_and_others); a
post-compile pass retargets the auto-inserted table loads to it and
drops ~50 redundant 1.3us reloads.

Attention: scores^T per head pair in one paired PSUM tile, one wide exp
(ACT).  v is augmented with 64 columns holding the key-padding mask, so
the att.v matmul leaves the softmax denominator replicated in PSUM rows
64:128: the drain is reciprocal + multiply (2 DVE ops), no broadcast
matmul / copies.  The av accumulation lags the score stream so the
PE never head-of-line blocks on exp.  The stream cadence is bound by
the scores->exp->slot-reuse chain around the 2-deep paired score ring,
so two key-tiles per stream (PMM_KTS) write their scores into two
1-bank pmm tiles instead (ring depth 3), two kts' exps run on DVE via
the Schraudolph bit trick (SCHRAUD_KTS), and v drains on ACT — putting
ACT/DVE/PE all near the same attention-phase occupancy.  All movable
PE work rides inside the streams: the remaining v-projection
half-tiles, the next pair's q/k groups, and the first query-half's
output projection (inside the last pair's second-half stream).
Sprinkled writes always precede their stream readers in program order
— the tile framework derives dependencies from program order, so a
late write is a race on hardware (CoreSim's race detector catches
this).

out-proj/ff2 drain with a fused (acc+bias)+residual STT op; bias
broadcast matmuls are gone.  Weight DMAs are batched (one qkw tensor
per layer, biases packed into one [P,48] tensor, ff1 in 6 chunks, ff2
one DMA per output tile reused across both query halves) so the sync
sequencer issues ~15 descriptors per layer instead of ~90; all DRAM
layouts are pre-arranged on the host so every DMA line is >=512B
contiguous per partition.  ff1 drains on ACT (Relu with per-partition
bias ptr) since DVE is the busier engine; ff2 runs in m-pairs with the
n=0 half first so the next layer's LN1 row chain + first-half
normalize hide behind the final n=1 matmul groups.
"""

import math
from collections import deque

import numpy as np

B, S, D, H, L, V, C = 8, 1024, 768, 12, 6, 32000, 2
HD, DF, MAXPOS = 64, 3072, 2048
P = 128
KT = D // P    # 6 feature tiles
NT = S // P    # 8 token tiles
FT = DF // P   # 24 ff tiles
FCH = 6        # ff1 weight chunks
FCM = FT // FCH  # 4 m-tiles per chunk
NQ = 2         # query halves of 512
QW = S // NQ   # 512
EPS = 1e-5
N_CORES = 8

_CACHE = {}
FP8_QKV = True      # q/k/v projections in fp8e4 DoubleRow (weights x32)
FP8_SCALE = 32.0
AV_FP8 = True       # att.v in fp8e4 DoubleRow (v and exp(scores) in fp8)
AV_SCALE = 4.0 if AV_FP8 else 1.0  # v pre-scale (pow2; mask cols match)
# Key-tiles whose exp runs on DVE instead of ACT via a Schraudolph-style
# direct-to-e4m3 bit trick: bits = round(s*8/ln2 + 55.5), written as uint8
# and bitcast to fp8e4.  Offloads 3/8 of the exp stream from the ACT
# bottleneck, and measures *better* on hardware than ACT exp + f8 cast
# (8.6e-3 vs 1.5e-2 rel err — the DVE f32->u8 convert rounds to nearest
# while the ACT f8 store appears to truncate).
SCHRAUD_KTS = (5, 7)
SCHRAUD_C1 = 8.0 / math.log(2.0)
SCHRAUD_C2 = 55.5
# Key-tiles whose score matmuls land in two 1-bank pmm tiles instead of the
# paired pscore tile: raises the effective score-ring depth from 2 to 3 so
# the scores->exp->reuse chain stops bounding the stream cadence.
PMM_KTS = (2, 5)

# packed bias layout: [qkb(12) | ob(6) | f1b(24) | f2b(6)]
BQK, BOB, BF1, BF2 = 0, 12, 18, 42
NBIAS = 48


def _build_nc(n_layers=L):
    import concourse.bass as bass
    import concourse.tile as tile
    from concourse import bacc, mybir
    from concourse.bass import ds, ts
    from concourse.masks import make_identity
    from contextlib import ExitStack

    f32 = mybir.dt.float32
    bf16 = mybir.dt.bfloat16
    f32r = mybir.dt.float32r
    f8 = mybir.dt.float8e4
    u8 = mybir.dt.uint8
    i32 = mybir.dt.int32
    wdt = f8 if FP8_QKV else bf16
    vdt = f8 if AV_FP8 else bf16
    DR = mybir.MatmulPerfMode.DoubleRow
    AF = mybir.ActivationFunctionType
    OP = mybir.AluOpType

    nc = bacc.Bacc("TRN2", target_bir_lowering=False, debug=False)

    # ---------------- DRAM I/O ----------------
    d_ids = nc.dram_tensor("ids", [P, NT], i32, kind="ExternalInput")
    d_gm = nc.dram_tensor("gmask", [P, NT], f32, kind="ExternalInput")
    d_emb = nc.dram_tensor("emb", [V, D], bf16, kind="ExternalInput")
    d_posT = nc.dram_tensor("posT", [D, S], bf16, kind="ExternalInput")
    d_qkw = nc.dram_tensor("qkw", [L, P, 12, KT, P], wdt, kind="ExternalInput")
    d_vw = nc.dram_tensor("vw", [L, P, KT, D], wdt, kind="ExternalInput")
    d_vb = nc.dram_tensor("vb", [L, D], bf16, kind="ExternalInput")
    d_bias = nc.dram_tensor("biases", [L, P, NBIAS], f32, kind="ExternalInput")
    d_ow = nc.dram_tensor("ow", [L, P, KT, KT, P], bf16, kind="ExternalInput")
    d_f1w = nc.dram_tensor("f1w", [L, FCH, P, FCM, KT, P], bf16,
                           kind="ExternalInput")
    d_f2w = nc.dram_tensor("f2w", [L, KT, P, FT, P], bf16, kind="ExternalInput")
    d_cw = nc.dram_tensor("cw", [P, KT, C], f32, kind="ExternalInput")
    d_cb = nc.dram_tensor("cb", [1, C], f32, kind="ExternalInput")
    d_out = nc.dram_tensor("out", [1, C], f32, kind="ExternalOutput")

    with tile.TileContext(nc) as tc, ExitStack() as ctx:
        # ---------------- pools ----------------
        state = ctx.enter_context(tc.tile_pool(name="state", bufs=1))
        consts = ctx.enter_context(tc.tile_pool(name="consts", bufs=1))
        b24 = ctx.enter_context(tc.tile_pool(name="b24", bufs=1))
        p48 = ctx.enter_context(tc.tile_pool(name="p48", bufs=1))
        vpool = ctx.enter_context(tc.tile_pool(name="vpool", bufs=1))
        attp = ctx.enter_context(tc.tile_pool(name="attp", bufs=1))
        vwpool = ctx.enter_context(tc.tile_pool(name="vwpool", bufs=1))
        qkwp = ctx.enter_context(tc.tile_pool(name="qkwp", bufs=2))
        w6 = ctx.enter_context(tc.tile_pool(name="w6", bufs=2))
        wff2 = ctx.enter_context(tc.tile_pool(name="wff2", bufs=2))
        epool = ctx.enter_context(tc.tile_pool(name="epool", bufs=3))
        tmp = ctx.enter_context(tc.tile_pool(name="tmp", bufs=4))
        tsub = ctx.enter_context(tc.tile_pool(name="tsub", bufs=5))
        mbrb = ctx.enter_context(tc.tile_pool(name="mbrb", bufs=2))
        srows = ctx.enter_context(tc.tile_pool(name="srows", bufs=3))
        srows1 = ctx.enter_context(tc.tile_pool(name="srows1", bufs=2))
        rows = ctx.enter_context(tc.tile_pool(name="rows", bufs=2))
        params = ctx.enter_context(tc.tile_pool(name="params", bufs=2))
        # PSUM budget: 8 banks = pscore 2x2 + pmm 2x1 + patt 2x1
        pscore = ctx.enter_context(tc.tile_pool(name="pscore", bufs=2, space="PSUM"))
        pmm = ctx.enter_context(tc.tile_pool(name="pmm", bufs=2, space="PSUM"))
        patt = ctx.enter_context(tc.tile_pool(name="patt", bufs=2, space="PSUM"))

        # ---------------- constants ----------------
        xT = state.tile([P, KT, S], bf16, tag="xT")
        ones_f32 = consts.tile([P, 1], f32, tag="ones_f32")
        nc.vector.memset(ones_f32[:, :], 1.0)
        ones_rf32 = consts.tile([1, QW], f32, tag="ones_rf32")
        nc.vector.memset(ones_rf32[:, :], 1.0)
        ones_col = consts.tile([P, 1], bf16, tag="ones_col")
        nc.vector.tensor_copy(out=ones_col[:, :], in_=ones_f32[:, :])
        ones_colr = consts.tile([P, 1], f32r, tag="ones_colr")
        nc.vector.tensor_copy(out=ones_colr[:, :], in_=ones_f32[:, :])
        ones_row = consts.tile([1, QW], f32r, tag="ones_row")
        nc.vector.tensor_copy(out=ones_row[:, :], in_=ones_rf32[:, :])
        ones_rbf = consts.tile([1, QW], bf16, tag="ones_rbf")
        nc.vector.tensor_copy(out=ones_rbf[:, :], in_=ones_rf32[:, :])
        ident = consts.tile([P, P], bf16, tag="ident")
        make_identity(nc, ident[:, :])
        eps_sb = consts.tile([1, 1], f32, tag="eps")
        nc.vector.memset(eps_sb[:, :], EPS)
        ids_sb = consts.tile([P, NT], i32, tag="ids")
        nc.sync.dma_start(out=ids_sb[:, :], in_=d_ids[:, :])
        gcol = consts.tile([P, NT], f32, tag="gcol")
        nc.sync.dma_start(out=gcol[:, :], in_=d_gm[:, :])
        gcol_s = consts.tile([P, NT], f32, tag="gcol_s")
        nc.vector.tensor_scalar(
            out=gcol_s[:, :], in0=gcol[:, :],
            scalar1=(AV_SCALE / FP8_SCALE if FP8_QKV else AV_SCALE), scalar2=None,
            op0=OP.mult,
        )
        gcol_m = consts.tile([P, NT], f32, tag="gcol_m")
        nc.vector.tensor_scalar(
            out=gcol_m[:, :], in0=gcol[:, :],
            scalar1=AV_SCALE, scalar2=None, op0=OP.mult,
        )
        cw_sb = consts.tile([P, KT, C], f32r, tag="cw")
        nc.sync.dma_start(out=cw_sb[:, :, :], in_=d_cw[:, :, :].bitcast(f32r))
        cb_sb = consts.tile([1, C], f32r, tag="cb")
        nc.sync.dma_start(out=cb_sb[:, :], in_=d_cb[:, :].bitcast(f32r))

        # persistent v tile: right half holds the key-padding mask column
        # replicated 64x (written once; av matmuls then leave the softmax
        # denominator replicated in psum rows 64:128)
        v_sb = vpool.tile([P, NT, H, 2 * HD], vdt, tag="v")
        for t in range(NT):
            nc.vector.tensor_copy(
                out=v_sb[:, t, :, HD : 2 * HD],
                in_=gcol_m[:, t : t + 1].to_broadcast([P, H, HD]),
            )

        # per-layer weight prefetch (one slot ahead via bufs=2 rings)
        layer_w = {}

        def issue_layer_weights(l):
            qkw_sb = qkwp.tile([P, 12, KT, P], wdt, tag="qkw", name="qkw_sb")
            nc.sync.dma_start(out=qkw_sb[:, :, :, :], in_=d_qkw[l])
            vw_sb = qkwp.tile([P, KT, D], wdt, tag="vw", name="vw_sb")
            nc.sync.dma_start(out=vw_sb[:, :, :], in_=d_vw[l])
            vb_row = rows.tile([1, D], bf16, tag="brow")
            nc.sync.dma_start(out=vb_row[:, :], in_=d_vb[l : l + 1, :])
            bias_sb = params.tile([P, NBIAS], f32, tag="bias")
            nc.sync.dma_start(out=bias_sb[:, :], in_=d_bias[l])
            layer_w[l] = (qkw_sb, vw_sb, vb_row, bias_sb)

        issue_layer_weights(0)

        # ---------------- embedding ----------------
        posT_sb = b24.tile([P, KT, S], bf16, tag="b24")
        nc.sync.dma_start(
            out=posT_sb[:, :, :], in_=d_posT.rearrange("(j p) s -> p j s", p=P)
        )
        embts = {}

        def emb_gather(t):
            embt = tmp.tile([P, D], bf16, tag="tmp")
            nc.gpsimd.indirect_dma_start(
                out=embt[:, :],
                out_offset=None,
                in_=d_emb[:, :],
                in_offset=bass.IndirectOffsetOnAxis(ap=ids_sb[:, t : t + 1], axis=0),
            )
            embts[t] = embt

        # 3-deep prefetch: never allocate a ring slot before its previous
        # occupant's readers have been emitted
        for t in range(3):
            emb_gather(t)
        for t in range(NT):
            if t + 3 < NT:
                emb_gather(t + 3)
            embt = embts.pop(t)
            tr = pmm.tile([P, KT, P], bf16, tag="pmm")
            for j in range(KT):
                nc.tensor.transpose(
                    out=tr[:, j, :],
                    in_=embt[:, j * P : (j + 1) * P],
                    identity=ident[:, :],
                )
            nc.vector.tensor_tensor(
                out=xT[:, :, ts(t, P)], in0=tr[:, :, :],
                in1=posT_sb[:, :, ts(t, P)], op=OP.add,
            )

        # ---------------- layer norm (split into stats / finish) ----------
        def ln_stats_start():
            st = pscore.tile([1, 2 * QW], f32, tag="ps", name="st")
            return st

        def ln_stats_step(st, src, n, j):
            nsl = ds(n * QW, QW)
            sq = tmp.tile([P, QW], f32r, tag="tmp")
            nc.scalar.square(sq[:, :], src[:, j, nsl])
            nc.tensor.matmul(
                st[:, 0:QW], ones_col[:, :], src[:, j, nsl],
                start=(j == 0), stop=(j == KT - 1),
            )
            nc.tensor.matmul(
                st[:, QW : 2 * QW], ones_colr[:, :], sq[:, :],
                start=(j == 0), stop=(j == KT - 1),
            )

        def ln_rows(st):
            """Mean/rstd row chain for one query half."""
            mean = srows1.tile([1, QW], f32r, tag="mean")
            nc.vector.tensor_scalar(
                out=mean[:, :], in0=st[:, 0:QW], scalar1=1.0 / D, scalar2=None,
                op0=OP.mult,
            )
            msq = srows.tile([1, QW], f32, tag="srow")
            nc.vector.tensor_scalar(
                out=msq[:, :], in0=st[:, QW : 2 * QW], scalar1=1.0 / D,
                scalar2=None, op0=OP.mult,
            )
            var = srows.tile([1, QW], f32, tag="srow")
            nc.vector.scalar_tensor_tensor(
                out=var[:, :], in0=mean[:, :], scalar=-1.0, in1=mean[:, :],
                op0=OP.mult, op1=OP.mult,
            )
            nc.vector.tensor_tensor(
                out=var[:, :], in0=var[:, :], in1=msq[:, :], op=OP.add,
            )
            lnv = srows.tile([1, QW], f32, tag="srow")
            nc.scalar.activation(lnv[:, :], var[:, :], AF.Ln, bias=eps_sb[:, :])
            rstd = srows1.tile([1, QW], f32r, tag="rstd")
            nc.scalar.activation(rstd[:, :], lnv[:, :], AF.Exp, scale=-0.5)
            return mean, rstd

        def ln_apply(mean_rstd, src, dst, n):
            """Broadcast mean/rstd across partitions and normalize."""
            mean, rstd = mean_rstd
            nsl = ds(n * QW, QW)
            bc = pscore.tile([P, 2, QW], f32, tag="ps", name="bc")
            nc.tensor.matmul(
                bc[:, 0, :], ones_row[0:1, 0:P], mean[:, :],
                start=True, stop=True,
            )
            nc.tensor.matmul(
                bc[:, 1, :], ones_row[0:1, 0:P], rstd[:, :],
                start=True, stop=True,
            )
            mr = mbrb.tile([P, 2, QW], bf16, tag="mbrb")
            nc.scalar.copy(out=mr[:, :, :], in_=bc[:, :, :])
            for j in range(KT):
                t1 = tsub.tile([P, QW], bf16, tag="tsub")
                nc.vector.tensor_tensor(
                    out=t1[:, :], in0=src[:, j, nsl], in1=mr[:, 0, :],
                    op=OP.subtract,
                )
                nc.vector.tensor_tensor(
                    out=dst[:, j, nsl], in0=t1[:, :], in1=mr[:, 1, :],
                    op=OP.mult,
                )

        # ---------------- layers ----------------
        ln1_st = {}
        for n in range(NQ):
            st = ln_stats_start()
            for j in range(KT):
                ln_stats_step(st, xT, n, j)
            ln1_st[n] = st

        ln1_rows = {}
        hT_next = None
        for l in range(n_layers):
            qkw_sb, vw_sb, vb_row, bias_sb = layer_w.pop(l)
            hT = hT_next if hT_next is not None else b24.tile(
                [P, KT, S], wdt, tag="b24", name="hT")
            hT_next = None
            qk_sb = p48.tile([P, 12, S], bf16, tag="p48")
            attT = attp.tile([P, KT, S], bf16, tag="attT")

            def v_proj_half(t, c0, cn):
                for c0, cn in ((c0, cn),):
                    acc = pmm.tile([P, QW], f32, tag="pmm", name="vacc")
                    nc.tensor.matmul(
                        acc[:, 0:cn], ones_rbf[0:1, 0:P], vb_row[:, c0 : c0 + cn],
                        start=True, stop=False,
                    )
                    if FP8_QKV:
                        for i in range(KT // 2):
                            nc.tensor.matmul(
                                acc[:, 0:cn],
                                hT[:, 2 * i : 2 * i + 2, ts(t, P)],
                                vw_sb[:, 2 * i : 2 * i + 2, c0 : c0 + cn],
                                start=False, stop=(i == KT // 2 - 1),
                                perf_mode=DR,
                            )
                    else:
                        for j in range(KT):
                            nc.tensor.matmul(
                                acc[:, 0:cn], hT[:, j, ts(t, P)],
                                vw_sb[:, j, c0 : c0 + cn],
                                start=False, stop=(j == KT - 1),
                            )
                    # drain on ACT (Copy with per-partition scale ptr) — DVE
                    # is the loaded engine during attention
                    nc.scalar.activation(
                        v_sb[:, t, c0 // HD : (c0 + cn) // HD, 0:HD],
                        acc[:, 0:cn].rearrange("p (h d) -> p h d", d=HD),
                        AF.Copy, scale=gcol_s[:, t : t + 1],
                    )

            def qk_group(m, n):
                nsl = ds(n * QW, QW)
                acc = pmm.tile([P, QW], f32, tag="pmm", name="qkacc")
                if FP8_QKV:
                    for i in range(KT // 2):
                        nc.tensor.matmul(
                            acc[:, :],
                            qkw_sb[:, m, 2 * i : 2 * i + 2, :],
                            hT[:, 2 * i : 2 * i + 2, nsl],
                            start=(i == 0), stop=(i == KT // 2 - 1),
                            perf_mode=DR,
                        )
                    nc.vector.tensor_scalar(
                        out=qk_sb[:, m, nsl], in0=acc[:, :],
                        scalar1=1.0 / FP8_SCALE,
                        scalar2=bias_sb[:, BQK + m : BQK + m + 1],
                        op0=OP.mult, op1=OP.add,
                    )
                else:
                    for j in range(KT):
                        nc.tensor.matmul(
                            acc[:, :], qkw_sb[:, m, j, :], hT[:, j, nsl],
                            start=(j == 0), stop=(j == KT - 1),
                        )
                    nc.vector.tensor_scalar(
                        out=qk_sb[:, m, nsl], in0=acc[:, :],
                        scalar1=bias_sb[:, BQK + m : BQK + m + 1], scalar2=None,
                        op0=OP.add,
                    )

            # LN1 finish + minimal serial prefix: only pair-0 q/k and the
            # first two v tiles run before the score streams; the remaining
            # v tiles and later pairs' q/k ride inside the streams (the PE
            # has slack there — the streams are ACT-exp-bound).
            def v_proj_t(t):
                v_proj_half(t, 0, QW)
                v_proj_half(t, QW, D - QW)

            if 0 in ln1_st:
                ln1_rows[0] = ln_rows(ln1_st.pop(0))
            if 0 in ln1_rows:
                ln_apply(ln1_rows.pop(0), xT, hT, 0)
            # n=0-half work first: hT n=0 is ready from the previous layer's
            # tail, so PE starts immediately; the n=1 row chain + normalize
            # run on ACT/DVE behind it
            v_proj_t(0)
            v_proj_t(1)
            qk_group(6, 0)
            qk_group(0, 0)
            if 1 in ln1_st:
                ln1_rows[1] = ln_rows(ln1_st.pop(1))
            if 1 in ln1_rows:
                ln_apply(ln1_rows.pop(1), xT, hT, 1)
            v_proj_t(2)
            v_proj_t(3)
            qk_group(6, 1)
            qk_group(0, 1)
            vq_queue = []
            for t in range(4, NT):
                vq_queue.append(("v", t, 0, QW))
                vq_queue.append(("v", t, QW, D - QW))

            # out-proj weights prefetched behind the attention stream
            ow_sb = vwpool.tile([P, KT, KT, P], bf16, tag="ow")
            nc.sync.dma_start(out=ow_sb[:, :, :, :], in_=d_ow[l])

            ln2_st = {}

            def out_proj_half(n, with_stats):
                nsl = ds(n * QW, QW)
                st2 = ln_stats_start() if with_stats else None
                for m in range(KT):
                    acc = pmm.tile([P, QW], f32, tag="pmm", name="oacc")
                    for j in range(KT):
                        nc.tensor.matmul(
                            acc[:, :], ow_sb[:, m, j, :], attT[:, j, nsl],
                            start=(j == 0), stop=(j == KT - 1),
                        )
                    nc.vector.scalar_tensor_tensor(
                        out=xT[:, m, nsl], in0=acc[:, :],
                        scalar=bias_sb[:, BOB + m : BOB + m + 1],
                        in1=xT[:, m, nsl],
                        op0=OP.add, op1=OP.add,
                    )
                    if with_stats:
                        ln_stats_step(st2, xT, n, m)
                if with_stats:
                    ln2_st[n] = st2

            for hp in range(H // 2):
                hA, hB = 2 * hp, 2 * hp + 1
                if hp + 1 < H // 2:
                    # k groups first: scores kt spans both halves of k
                    vq_queue += [("qk", 7 + hp, 0), ("qk", 7 + hp, 1),
                                 ("qk", 1 + hp, 0), ("qk", 1 + hp, 1)]
                pend = deque()
                pats = {}

                if AV_FP8:
                    def av_emit(n, ktp, epair):
                        if ktp == 0:
                            patA = patt.tile([P, QW], f32, tag="patt", name="patA")
                            patB = patt.tile([P, QW], f32, tag="patt", name="patB")
                            pats[n] = (patA, patB)
                        pA, pB = pats[n]
                        nc.tensor.matmul(
                            pA[:, :], v_sb[:, 2 * ktp : 2 * ktp + 2, hA, :],
                            epair[:, :, 0:QW],
                            start=(ktp == 0), stop=(ktp == NT // 2 - 1),
                            perf_mode=DR,
                        )
                        nc.tensor.matmul(
                            pB[:, :], v_sb[:, 2 * ktp : 2 * ktp + 2, hB, :],
                            epair[:, :, QW : 2 * QW],
                            start=(ktp == 0), stop=(ktp == NT // 2 - 1),
                            perf_mode=DR,
                        )
                else:
                    def av_emit(n, kt, e):
                        if kt == 0:
                            patA = patt.tile([P, QW], f32, tag="patt", name="patA")
                            patB = patt.tile([P, QW], f32, tag="patt", name="patB")
                            pats[n] = (patA, patB)
                        pA, pB = pats[n]
                        nc.tensor.matmul(
                            pA[:, :], v_sb[:, kt, hA, :], e[:, 0:QW],
                            start=(kt == 0), stop=(kt == NT - 1),
                        )
                        nc.tensor.matmul(
                            pB[:, :], v_sb[:, kt, hB, :], e[:, QW : 2 * QW],
                            start=(kt == 0), stop=(kt == NT - 1),
                        )

                for n in range(NQ):
                    nsl = ds(n * QW, QW)
                    epair = None
                    for kt in range(NT):
                        if AV_FP8 and kt in PMM_KTS:
                            # split scores into two 1-bank pmm tiles: deepens
                            # the score ring so the scores->exp->slot-reuse
                            # chain stops pacing the stream
                            pieces = []
                            for h0, c0 in ((0, 0), (HD, QW)):
                                p1 = pmm.tile([P, QW], f32, tag="pmm",
                                              name="pssplit")
                                nc.tensor.matmul(
                                    p1[:, :],
                                    qk_sb[h0 : h0 + HD, 6 + hp, ts(kt, P)],
                                    qk_sb[h0 : h0 + HD, hp, nsl],
                                    start=True, stop=True,
                                )
                                pieces.append((p1, c0, QW))
                        else:
                            ps = pscore.tile([P, 2 * QW], f32, tag="ps")
                            nc.tensor.matmul(
                                ps[:, 0:QW],
                                qk_sb[0:HD, 6 + hp, ts(kt, P)],
                                qk_sb[0:HD, hp, nsl],
                                start=True, stop=True,
                            )
                            nc.tensor.matmul(
                                ps[:, QW : 2 * QW],
                                qk_sb[HD:P, 6 + hp, ts(kt, P)],
                                qk_sb[HD:P, hp, nsl],
                                start=True, stop=True,
                            )
                            pieces = [(ps, 0, 2 * QW)]
                        if AV_FP8:
                            if kt % 2 == 0:
                                epair = epool.tile([P, 2, 2 * QW], f8, tag="e",
                                                   name="epair")
                            for src, c0, cw in pieces:
                                dst = epair[:, kt % 2, c0 : c0 + cw]
                                if kt in SCHRAUD_KTS:
                                    nc.vector.tensor_scalar(
                                        out=dst.bitcast(u8),
                                        in0=src[:, 0:cw], scalar1=SCHRAUD_C1,
                                        scalar2=SCHRAUD_C2,
                                        op0=OP.mult, op1=OP.add,
                                    )
                                else:
                                    nc.scalar.activation(
                                        dst, src[:, 0:cw], AF.Exp)
                            if kt % 2 == 1:
                                pend.append((n, kt // 2, epair))
                                if len(pend) >= 2:
                                    av_emit(*pend.popleft())
                        else:
                            ps = pieces[0][0]
                            e = epool.tile([P, 2 * QW], bf16, tag="e")
                            nc.scalar.activation(e[:, :], ps[:, :], AF.Exp)
                            pend.append((n, kt, e))
                            if len(pend) >= 3:
                                av_emit(*pend.popleft())
                        if 1 <= kt <= 6 and vq_queue:
                            # two pops when both are v-halves: the first
                            # stream must emit every v write before the av
                            # that reads it (program order = dependency
                            # order for the tile framework)
                            for _ in range(2 if vq_queue[0][0] == "v" else 1):
                                if not vq_queue:
                                    break
                                task = vq_queue.pop(0)
                                if task[0] == "v":
                                    v_proj_half(task[1], task[2], task[3])
                                else:
                                    qk_group(task[1], task[2])
                    while pend:
                        av_emit(*pend.popleft())
                    # drain: denominator is replicated in psum rows 64:128
                    pA, pB = pats.pop(n)
                    for pat, po in ((pA, 0), (pB, HD)):
                        zinv = srows.tile([HD, QW], bf16, tag="zinv")
                        with nc.allow_low_precision(reason="softmax denom bf16"):
                            nc.vector.reciprocal(zinv[:, :], pat[HD:P, :])
                        nc.vector.tensor_tensor(
                            out=attT[po : po + HD, hp, nsl],
                            in0=pat[0:HD, :], in1=zinv[:, :], op=OP.mult,
                        )
                    if hp == H // 2 - 1 and n == 0:
                        # n=0 attention fully drained: overlap the n=0
                        # out-projection with the last pair's n=1 stream
                        out_proj_half(0, with_stats=False)
                while vq_queue:
                    task = vq_queue.pop(0)
                    if task[0] == "v":
                        v_proj_half(task[1], task[2], task[3])
                    else:
                        qk_group(task[1], task[2])

            # ---- output projection + residual, n-split, LN2 stats behind;
            # the n=0 half was emitted inside the last head pair's n=1
            # attention stream ----
            st2 = ln_stats_start()
            for m in range(KT):
                ln_stats_step(st2, xT, 0, m)
            ln2_st[0] = st2
            # n=0 row chain + normalize issued before out_proj(1) so all
            # their ACT/DVE latency hides behind those matmuls and ff1 can
            # start the moment out_proj(1) drains
            r0 = ln_rows(ln2_st.pop(0))
            h2 = b24.tile([P, KT, S], bf16, tag="b24")
            ln_apply(r0, xT, h2, 0)
            out_proj_half(1, with_stats=True)
            r1 = ln_rows(ln2_st.pop(1))
            ln_apply(r1, xT, h2, 1)

            f1w_tiles = {}

            def dma_f1w(ch):
                wt = w6.tile([P, FCM, KT, P], bf16, tag="w6", name="f1wc")
                nc.sync.dma_start(out=wt[:, :, :, :], in_=d_f1w[l, ch])
                f1w_tiles[ch] = wt

            f2w_tiles = {}

            def dma_f2w(m):
                w2 = wff2.tile([P, FT, P], bf16, tag="wff2", name="f2wt")
                nc.sync.dma_start(out=w2[:, :, :], in_=d_f2w[l, m])
                f2w_tiles[m] = w2

            dma_f1w(0)
            dma_f1w(1)
            if l + 1 < n_layers:
                issue_layer_weights(l + 1)
            dma_f2w(0)
            dma_f2w(1)

            f_sb = p48.tile([P, FT, S], bf16, tag="p48")
            for ch in range(FCH):
                if ch >= 1 and ch + 1 < FCH:
                    dma_f1w(ch + 1)
                wt = f1w_tiles.pop(ch)
                # n-major within the chunk: the first chunk's n=0 groups can
                # start as soon as the n=0 normalize lands, hiding the n=1
                # apply latency behind them
                for n in range(NQ):
                    for mi in range(FCM):
                        m = ch * FCM + mi
                        nsl = ds(n * QW, QW)
                        acc = pmm.tile([P, QW], f32, tag="pmm", name="facc")
                        for j in range(KT):
                            nc.tensor.matmul(
                                acc[:, :], wt[:, mi, j, :], h2[:, j, nsl],
                                start=(j == 0), stop=(j == KT - 1),
                            )
                        # drain on ACT (idle during FFN; DVE is the busy one)
                        nc.scalar.activation(
                            f_sb[:, m, nsl], acc[:, :], AF.Relu,
                            bias=bias_sb[:, BF1 + m : BF1 + m + 1],
                        )

            last = l == n_layers - 1
            st1 = {} if last else {n: ln_stats_start() for n in range(NQ)}

            def ff2_half(m, n):
                nsl = ds(n * QW, QW)
                w2 = f2w_tiles[m]
                acc = pmm.tile([P, QW], f32, tag="pmm", name="f2acc")
                for j in range(FT):
                    nc.tensor.matmul(
                        acc[:, :], w2[:, j, :], f_sb[:, j, nsl],
                        start=(j == 0), stop=(j == FT - 1),
                    )
                nc.vector.scalar_tensor_tensor(
                    out=xT[:, m, nsl], in0=acc[:, :],
                    scalar=bias_sb[:, BF2 + m : BF2 + m + 1],
                    in1=xT[:, m, nsl],
                    op0=OP.add, op1=OP.add,
                )
                if not last:
                    ln_stats_step(st1[n], xT, n, m)

            # m-pairs with n=0 first within each pair: st1[0] closes two
            # groups before the end, so the next layer's first-half
            # normalize hides behind the final n=1 groups
            order = [(0, 0), (1, 0), (0, 1), (1, 1), (2, 0), (3, 0),
                     (2, 1), (3, 1), (4, 0), (5, 0), (4, 1), (5, 1)]
            done_n = {m: 0 for m in range(KT)}
            for m, n in order:
                ff2_half(m, n)
                done_n[m] += 1
                if done_n[m] == NQ:
                    f2w_tiles.pop(m)
                    if m + 2 < KT:
                        dma_f2w(m + 2)
                if (m, n) == (KT - 1, 0) and not last:
                    ln1_rows[0] = ln_rows(st1.pop(0))
                    hT_next = b24.tile([P, KT, S], wdt, tag="b24", name="hTn")
                    ln_apply(ln1_rows.pop(0), xT, hT_next, 0)
            if not last:
                ln1_st[1] = st1[1]

        # ---------------- CLS head ----------------
        col2 = xT[:, :, 0:2]  # (P, KT, 2) bf16
        xsqc = consts.tile([P, KT, 2], bf16, tag="xsqc")
        nc.scalar.square(xsqc[:, :, :], col2)
        pss = pmm.tile([1, QW], f32, tag="pmm")
        for j in range(KT):
            nc.tensor.matmul(
                pss[:, 0:2], ones_col[:, :], xT[:, j, 0:2],
                start=(j == 0), stop=(j == KT - 1),
            )
        for j in range(KT):
            nc.tensor.matmul(
                pss[:, 2:4], ones_col[:, :], xsqc[:, j, :],
                start=(j == 0), stop=(j == KT - 1),
            )
        hmean = srows1.tile([1, 64], f32r, tag="mean")
        nc.vector.tensor_scalar(
            out=hmean[:, 0:2], in0=pss[:, 0:2], scalar1=1.0 / D, scalar2=None,
            op0=OP.mult,
        )
        hmsq = srows.tile([1, 64], f32, tag="srow")
        nc.vector.tensor_scalar(
            out=hmsq[:, 0:2], in0=pss[:, 2:4], scalar1=1.0 / D, scalar2=None,
            op0=OP.mult,
        )
        hvar = srows.tile([1, 64], f32, tag="srow")
        nc.vector.scalar_tensor_tensor(
            out=hvar[:, 0:2], in0=hmean[:, 0:2], scalar=-1.0, in1=hmean[:, 0:2],
            op0=OP.mult, op1=OP.mult,
        )
        nc.vector.tensor_tensor(
            out=hvar[:, 0:2], in0=hvar[:, 0:2], in1=hmsq[:, 0:2], op=OP.add
        )
        hlnv = srows.tile([1, 64], f32, tag="srow")
        nc.scalar.activation(hlnv[:, 0:2], hvar[:, 0:2], AF.Ln, bias=eps_sb[:, :])
        hrstd = srows1.tile([1, 64], f32r, tag="rstd")
        nc.scalar.activation(hrstd[:, 0:2], hlnv[:, 0:2], AF.Exp, scale=-0.5)
        pbc = pmm.tile([P, QW], f32, tag="pmm")
        nc.tensor.matmul(pbc[:, 0:2], ones_row[0:1, 0:P], hmean[:, 0:2],
                         start=True, stop=True)
        nc.tensor.matmul(pbc[:, 2:4], ones_row[0:1, 0:P], hrstd[:, 0:2],
                         start=True, stop=True)
        t1 = consts.tile([P, KT, 2], f32, tag="ht1")
        nc.vector.tensor_tensor(
            out=t1[:, :, :], in0=col2, in1=pbc[:, 0:1].to_broadcast([P, KT, 2]),
            op=OP.subtract,
        )
        pc = consts.tile([P, KT, 2], f32r, tag="pc")
        nc.vector.tensor_tensor(
            out=pc[:, :, :], in0=t1[:, :, :], in1=pbc[:, 2:3].to_broadcast([P, KT, 2]),
            op=OP.mult,
        )
        plog = patt.tile([P, QW], f32, tag="patt")
        nc.tensor.matmul(
            plog[0:C, 0:2], cb_sb[:, :], ones_row[:, 0:2], start=True, stop=False
        )
        for j in range(KT):
            nc.tensor.matmul(
                plog[0:C, 0:2], cw_sb[:, j, :], pc[:, j, :],
                start=False, stop=(j == KT - 1),
            )
        out_sb = consts.tile([C, 1], f32, tag="outsb")
        nc.vector.tensor_copy(out=out_sb[:, :], in_=plog[0:C, 0:1])
        nc.sync.dma_start(out=d_out[0:1, 0:C], in_=out_sb[0:C, 0:1])

    nc.compile()
    _patch_act_tables(nc)
    return nc


def _patch_act_tables(nc):
    """All activation functions used here (exp, ln, square, copy, relu,
    identity) live together in act func set 6 (natural_log_exp_and_others),
    but the table-load pass picks the first set containing each function,
    inserting ~50 1.3us reloads.  Retarget the first load to set 6 and drop
    the redundant ones (they carry no semaphore info)."""
    from concourse import mybir

    first = True
    for b in nc.m.functions[0].blocks:
        keep = []
        for inst in b.instructions:
            if isinstance(inst, mybir.InstLoadActFuncSet):
                si = inst.sync_info
                has_sems = si is not None and (
                    len(si.on_wait) > 0 or len(si.on_update) > 0
                )
                if first or has_sems:
                    inst.act_func_set_id = 6
                    keep.append(inst)
                    first = False
            else:
                keep.append(inst)
        b.instructions[:] = keep


def _bf16np():
    import ml_dtypes

    return ml_dtypes.bfloat16


def _f8np():
    import ml_dtypes

    return ml_dtypes.float8_e4m3fn


def _prep_host(inputs):
    g = lambda k: np.asarray(inputs[k])
    bf = _bf16np()
    sq = np.float32(math.sqrt(D))
    ids = g("input_ids").astype(np.int32)              # (B, S)
    gm = (1.0 - g("attention_mask").astype(np.float32))  # (B, S)
    emb = (g("token_emb").astype(np.float32) * sq)
    posT = np.ascontiguousarray((g("pos_emb")[:S].astype(np.float32) * sq).T)
    # reference reshapes qkv output to (H, 3, HD): permute columns into
    # contiguous q | k | v blocks (each h-major) before tiling
    idx = np.arange(3 * D).reshape(H, 3, HD)
    cols = np.concatenate(
        [idx[:, 0, :].reshape(-1), idx[:, 1, :].reshape(-1), idx[:, 2, :].reshape(-1)]
    )
    qkv_w_orig = g("qkv_w").astype(np.float32)[:, :, cols].copy()  # (L, D, 3D)
    qkv_b = g("qkv_b").astype(np.float32)[:, cols].copy()          # (L, 3D)
    qkv_w_orig[:, :, :D] *= np.float32(1.0 / math.sqrt(HD))
    qkv_b[:, :D] *= np.float32(1.0 / math.sqrt(HD))
    # fold LN1 scale/bias into qkv:  (x*s+b) @ W = x @ (diag(s)W) + (b@W)
    n1_s = g("n1_s").astype(np.float32)   # (L, D)
    n1_b = g("n1_b").astype(np.float32)
    qkv_w = qkv_w_orig * n1_s[:, :, None]
    qkv_b = qkv_b + np.einsum("ld,lde->le", n1_b, qkv_w_orig)
    # fold LN2 into ff1
    n2_s = g("n2_s").astype(np.float32)
    n2_b = g("n2_b").astype(np.float32)
    ff1_w_orig = g("ff1_w").astype(np.float32)          # (L, D, DF)
    ff1_w = ff1_w_orig * n2_s[:, :, None]
    ff1_b = g("ff1_b").astype(np.float32) + np.einsum("ld,ldf->lf", n2_b, ff1_w_orig)
    # fold head LN into cls
    hln_s = g("hln_s").astype(np.float32)
    hln_b = g("hln_b").astype(np.float32)
    cls_w_orig = g("cls_w").astype(np.float32)          # (D, C)
    cls_w = cls_w_orig * hln_s[:, None]
    cls_b = g("cls_b").astype(np.float32) + hln_b @ cls_w_orig

    wnp = _f8np() if FP8_QKV else bf
    qw_scale = np.float32(FP8_SCALE) if FP8_QKV else np.float32(1.0)
    # packed per-layer biases: [qkb(12) | ob(6) | f1b(24) | f2b(6)] as [L,P,48]
    qkb_t = qkv_b[:, : 2 * D].reshape(L, 12, P).transpose(0, 2, 1)
    ob_t = g("out_b").astype(np.float32).reshape(L, KT, P).transpose(0, 2, 1)
    f1b_t = ff1_b.reshape(L, FT, P).transpose(0, 2, 1)
    f2b_t = g("ff2_b").astype(np.float32).reshape(L, KT, P).transpose(0, 2, 1)
    biases = np.ascontiguousarray(
        np.concatenate([qkb_t, ob_t, f1b_t, f2b_t], axis=2)
    )
    shared = {
        "emb": emb.astype(bf),
        "posT": posT.astype(bf),
        "qkw": np.ascontiguousarray(
            (qkv_w[:, :, : 2 * D] * qw_scale)
            .reshape(L, KT, P, 12, P).transpose(0, 2, 3, 1, 4)
        ).astype(wnp),
        "vw": np.ascontiguousarray(
            (qkv_w[:, :, 2 * D :] * qw_scale)
            .reshape(L, KT, P, D).transpose(0, 2, 1, 3)
        ).astype(wnp),
        "vb": np.ascontiguousarray(qkv_b[:, 2 * D :] * qw_scale).astype(bf),
        "biases": biases,
        "ow": np.ascontiguousarray(
            g("out_w").astype(np.float32).reshape(L, KT, P, KT, P).transpose(0, 2, 3, 1, 4)
        ).astype(bf),
        "f1w": np.ascontiguousarray(
            ff1_w.reshape(L, KT, P, FT, P).transpose(0, 3, 2, 1, 4)
            .reshape(L, FCH, FCM, P, KT, P).transpose(0, 1, 3, 2, 4, 5)
        ).astype(bf),
        "f2w": np.ascontiguousarray(
            g("ff2_w").astype(np.float32).reshape(L, FT, P, KT, P).transpose(0, 3, 2, 1, 4)
        ).astype(bf),
        "cw": np.ascontiguousarray(cls_w.reshape(KT, P, C).transpose(1, 0, 2)),
        "cb": cls_b.reshape(1, C),
    }
    per_core = []
    for c in range(N_CORES):
        per_core.append(
            {
                "ids": np.ascontiguousarray(ids[c].reshape(NT, P).T),
                "gmask": np.ascontiguousarray(gm[c].reshape(NT, P).T),
            }
        )
    return shared, per_core


def _get_nc():
    if "nc" not in _CACHE:
        _CACHE["nc"] = _build_nc()
    return _CACHE["nc"]


def kernel(**inputs):
    from concourse.bass_utils import run_bass_kernel_spmd

    shared, per_core = _prep_host(inputs)
    nc = _get_nc()
    in_maps = [dict(shared, **per_core[c]) for c in range(N_CORES)]
    _CACHE["in_maps"] = in_maps
    res = run_bass_kernel_spmd(nc, in_maps, list(range(N_CORES)))
    out = np.stack([res.results[c]["out"][0] for c in range(N_CORES)], axis=0)
    return out.astype(np.float32)


def bench(n_iters=10):
    """Re-run the compiled NEFF with device-resident inputs; returns the
    best-observed per-iteration wall time in ns (upper bound on HW exec)."""
    import time

    import jax
    import numpy as _np
    from jax.sharding import Mesh, PartitionSpec, NamedSharding
    from jax.experimental.shard_map import shard_map
    from concourse import bass2jax, mybir
    from concourse.bass2jax import _bass_exec_p, install_neuronx_cc_hook

    nc = _get_nc()
    in_maps = _CACHE["in_maps"]
    install_neuronx_cc_hook()

    pname = nc.partition_id_tensor.name if nc.partition_id_tensor else None
    in_names, out_names, out_avals, zero_outs = [], [], [], []
    for alloc in nc.m.functions[0].allocations:
        if not isinstance(alloc, mybir.MemoryLocationSet):
            continue
        name = alloc.memorylocations[0].name
        if alloc.kind == "ExternalInput":
            if name == pname:
                continue
            in_names.append(name)
        elif alloc.kind == "ExternalOutput":
            out_names.append(name)
            shape = tuple(alloc.tensor_shape)
            dtype = mybir.dt.np(alloc.dtype)
            out_avals.append(jax.core.ShapedArray(shape, dtype))
            zero_outs.append(_np.zeros(shape, dtype))
    n_params = len(in_names)
    all_names = in_names + out_names
    if pname is not None:
        all_names = all_names + [pname]

    def _body(*args):
        operands = list(args)
        if pname is not None:
            operands.append(bass2jax.partition_id_tensor())
        outs = _bass_exec_p.bind(
            *operands,
            out_avals=tuple(out_avals),
            in_names=tuple(all_names),
            out_names=tuple(out_names),
            lowering_input_output_aliases=(),
            sim_require_finite=True,
            sim_require_nnan=True,
            nc=nc,
        )
        return tuple(outs)

    devices = jax.devices()[:N_CORES]
    mesh = Mesh(_np.asarray(devices), ("core",))
    nin = n_params + len(zero_outs)
    fn = jax.jit(
        shard_map(
            _body,
            mesh=mesh,
            in_specs=(PartitionSpec("core"),) * nin,
            out_specs=(PartitionSpec("core"),) * len(out_names),
            check_rep=False,
        )
    )
    sharding = NamedSharding(mesh, PartitionSpec("core"))
    concat_in = [
        jax.device_put(
            _np.concatenate([_np.asarray(in_maps[c][n]) for c in range(N_CORES)], 0),
            sharding,
        )
        for n in in_names
    ]
    concat_zeros = [
        jax.device_put(
            _np.zeros((N_CORES * z.shape[0], *z.shape[1:]), z.dtype), sharding
        )
        for z in zero_outs
    ]
    jax.block_until_ready(concat_in)
    # warmup (compile)
    out = fn(*concat_in, *concat_zeros)
    jax.block_until_ready(out)
    # pipelined async dispatch amortizes the axon tunnel round-trip
    outs = []
    t0 = time.perf_counter()
    for _ in range(n_iters):
        outs.append(fn(*concat_in, *concat_zeros))
    jax.block_until_ready(outs)
    dt = (time.perf_counter() - t0) / n_iters
    return int(dt * 1e9)
